# revision 23
# baseline (speedup 1.0000x reference)
"""Expert-parallel top-2 MoE kernel for 8 Trainium2 NeuronCores.

Strategy (expert-parallel, sparse dispatch, per the sharding hint):
  - Router sharded over cores: core c computes fp32 logits for its 512-token
    shard on the TensorEngine (Wg stationary), AllGathers them (contiguous
    [E, tok] layout, transposed back on the PE) so every core holds identical
    logits for all 4096 tokens; softmax/top-2 on-device.
  - Core c owns expert c: slot positions come from matmul-based exclusive
    cumsums; the slot->token map is built with a single indirect-scatter DMA
    (token ids scattered to their slot positions; unrouted tokens get an
    out-of-bounds position and are dropped via bounds_check). Routed tokens
    are gathered via indirect DMA, transposed on the TensorEngine, and run
    through the two-layer FFN in bf16 (capacity padded to a multiple of 384).
  - Unscaled expert outputs (bf16) are AllGathered chunk-by-chunk into one
    shared [NG*8*CHK, D] tensor (overlapped with the FFN); each core combines
    the top-2 contributions for its own 512-token shard with two indirect
    row-gathers per 128-token tile + gate-weighted sum in fp32.

Numerics: router fp32 (top-2 selection fidelity), FFN bf16 with fp32
accumulation in PSUM, combine in fp32.
"""

import os
import sys

import numpy as np

for _p in ("/opt/trn_rl_repo",):
    if _p not in sys.path:
        sys.path.append(_p)

import ml_dtypes

import concourse.bass as bass
import concourse.mybir as mybir
import concourse.tile as tile
from concourse import bacc
from concourse.bass import IndirectOffsetOnAxis
from concourse.masks import make_identity

# Problem shapes (fixed per spec)
B, S, D, E = 2, 2048, 1024, 8
T = B * S          # 4096 tokens
F = 4 * D          # 4096 ffn dim
P = 128            # partitions
NT = T // P        # 32 token tiles
KD = D // P        # 8 contraction tiles over D
NF = F // P        # 32 f tiles
TOK_PER_CORE = T // E   # 512
OWN_TILES = TOK_PER_CORE // P  # 4
N_CORES = E
TG = 3                       # slot tiles per FFN group
CHK = TG * P                 # 384: slot chunk for the chunked AllGather
BIGPOS = float(1 << 20)      # scatter position for unrouted tokens (dropped)

f32 = mybir.dt.float32
bf16 = mybir.dt.bfloat16
i32 = mybir.dt.int32
u32 = mybir.dt.uint32

_cache = {}


def build_module(C: int, debug_out: bool = False):
    """Build the SPMD Bass module for capacity C (multiple of 384)."""
    assert C % CHK == 0
    ST = C // P  # slot tiles per expert

    nc = bacc.Bacc("TRN2", target_bir_lowering=False, debug=False,
                   num_devices=N_CORES)

    # ---- I/O ----
    xTs = nc.dram_tensor("xTs", [D, TOK_PER_CORE], f32,
                         kind="ExternalInput").ap()
    xbf = nc.dram_tensor("xbf", [T, D], bf16, kind="ExternalInput").ap()
    w1d = nc.dram_tensor("w1d", [D, F], bf16, kind="ExternalInput").ap()
    w2d = nc.dram_tensor("w2d", [F, D], bf16, kind="ExternalInput").ap()
    wgd = nc.dram_tensor("wgd", [D, E], f32, kind="ExternalInput").ap()
    bgb = nc.dram_tensor("bgb", [P, NT * E], f32, kind="ExternalInput").ap()
    b1pm = nc.dram_tensor("b1pm", [P, NF], f32, kind="ExternalInput").ap()
    b2pm = nc.dram_tensor("b2pm", [P, D], f32, kind="ExternalInput").ap()
    sel256 = nc.dram_tensor("sel256", [P, NT * E], f32,
                            kind="ExternalInput").ap()
    l128d = nc.dram_tensor("l128d", [P, P], f32, kind="ExternalInput").ap()
    ownmd = nc.dram_tensor("ownmd", [P, OWN_TILES * NT], f32,
                           kind="ExternalInput").ap()
    out = nc.dram_tensor("out", [TOK_PER_CORE, D], f32,
                         kind="ExternalOutput").ap()
    dbg = None
    if debug_out:
        dbg = {
            "dbg_l": nc.dram_tensor("dbg_l", [P, NT * E], f32,
                                    kind="ExternalOutput").ap(),
            "dbg_pos": nc.dram_tensor("dbg_pos", [P, NT * E], f32,
                                      kind="ExternalOutput").ap(),
            "dbg_posm": nc.dram_tensor("dbg_posm", [P, NT], f32,
                                       kind="ExternalOutput").ap(),
            "dbg_idx": nc.dram_tensor("dbg_idx", [P, C // P], i32,
                                      kind="ExternalOutput").ap(),
            "dbg_xgT": nc.dram_tensor("dbg_xgT", [P, C], bf16,
                                      kind="ExternalOutput").ap(),
            "dbg_y": nc.dram_tensor("dbg_y", [C, D], bf16,
                                    kind="ExternalOutput").ap(),
            "dbg_S": nc.dram_tensor("dbg_S", [P, 4 * (C // CHK) * NT], f32,
                                    kind="ExternalOutput").ap(),
        }

    with tile.TileContext(nc) as tc:
        _emit(tc, C, ST, xTs, xbf, w1d, w2d, wgd, bgb, b1pm, b2pm, sel256,
              l128d, ownmd, out, dbg)

    nc.compile()
    return nc


def _emit(tc, C, ST, xTs, xbf, w1d, w2d, wgd, bgb, b1pm, b2pm, sel256,
          l128d, ownmd, out, dbg=None):
    nc = tc.nc
    NE = NT * E  # 256
    NG = ST // TG  # number of FFN groups / y-AllGather chunks
    GBLK = N_CORES * CHK  # rows per AllGather chunk in y_big

    # ---------------- persistent pools ----------------
    persist = tc.alloc_tile_pool(name="persist", bufs=1)
    dram = tc.alloc_tile_pool(name="dram", bufs=1, space="DRAM")

    # tiny warmup AllGather: absorbs first-collective setup cost and aligns
    # the cores before the logits AllGather on the critical path
    wup_in = dram.tile([E, 4], f32, name="wup_in")
    wup_out = dram.tile([N_CORES * E, 4], f32, addr_space="Shared",
                        name="wup_out")
    nc.gpsimd.collective_compute(
        "AllGather", mybir.AluOpType.bypass,
        replica_groups=[list(range(N_CORES))],
        ins=[wup_in[:].opt()], outs=[wup_out[:].opt()],
    )

    # constants / weights resident in SBUF
    wg_sb = persist.tile([P, KD, E], f32, name="wg_sb")
    nc.sync.dma_start(wg_sb[:], wgd.rearrange("(k p) e -> p k e", p=P))
    bg_sb = persist.tile([P, NE], f32, name="bg_sb")
    nc.sync.dma_start(bg_sb[:], bgb[:])
    sel_sb = persist.tile([P, NE], f32, name="sel_sb")
    nc.sync.dma_start(sel_sb[:], sel256[:])
    l128_sb = persist.tile([P, P], f32, name="l128_sb")
    nc.sync.dma_start(l128_sb[:], l128d[:])
    ownm_sb = persist.tile([P, OWN_TILES * NT], f32, name="ownm_sb")
    nc.sync.dma_start(ownm_sb[:], ownmd[:])
    b1_sb = persist.tile([P, NF], f32, name="b1_sb")
    nc.sync.dma_start(b1_sb[:], b1pm[:])
    b2_sb = persist.tile([P, D], f32, name="b2_sb")
    nc.sync.dma_start(b2_sb[:], b2pm[:])
    ident = persist.tile([P, P], bf16, name="ident")
    make_identity(nc, ident[:])
    identf = persist.tile([P, P], f32, name="identf")
    make_identity(nc, identf[:])
    ones_col = persist.tile([P, 1], f32, name="ones_col")
    nc.vector.memset(ones_col[:], 1.0)
    ones_row = persist.tile([1, P], f32, name="ones_row")
    nc.vector.memset(ones_row[:], 1.0)

    # w1 tiles allocated here; the 8MB of loads are issued after the router's
    # small DMAs so they don't head-of-line block the critical path
    w1_sb = [persist.tile([P, F], bf16, name=f"w1_sb{k}") for k in range(KD)]

    # router / dispatch state kept for the combine phase
    exp_all = persist.tile([P, NE], f32, name="exp_all")    # exp(logits)
    m8_all = persist.tile([P, NE], f32, name="m8_all")      # per-tile top8 of exp
    r_all = persist.tile([P, NT], f32, name="r_all")        # 1/sum(exp)
    pos_all = persist.tile([P, NE], f32, name="pos_all")    # excl cumsum per expert
    ind_all = persist.tile([P, NE], f32, name="ind_all")    # top2 indicator
    ei_all = persist.tile([P, NE], u32, name="ei_all")      # top8 expert indices
    idx_sb = persist.tile([P, ST], i32, name="idx_sb")      # slot -> token id

    xgT = [persist.tile([P, C], bf16, name=f"xgT{d}") for d in range(KD)]

    l_dram = dram.tile([E, TOK_PER_CORE], f32, name="l_dram")
    lg_dram = dram.tile([N_CORES * E, TOK_PER_CORE], f32, addr_space="Shared",
                        name="lg_dram")
    # 8 partial slot maps so the scatters don't serialize on WAW hazards
    smaps = [dram.tile([C, 1], i32, name=f"smap{k}") for k in range(8)]
    y_dram = [dram.tile([CHK, D], bf16, name=f"y_dram{g}") for g in range(NG)]
    # one Shared AllGather output per chunk (Shared tensors allow one writer)
    y_all = [dram.tile([N_CORES * CHK, D], bf16, addr_space="Shared",
                       name=f"y_all{g}") for g in range(NG)]

    # ---------------- router (sharded + AllGather) ----------------
    with tc.tile_pool(name="router_sb", bufs=1, named_scope="router") as rpool, \
         tc.tile_pool(name="router_ps", bufs=1, space="PSUM") as rps:
        xs = rpool.tile([P, KD, TOK_PER_CORE], f32, name="xs")
        nc.sync.dma_start(xs[:], xTs.rearrange("(k p) t -> p k t", p=P))
        lT = rps.tile([E, TOK_PER_CORE], f32, name="lT")
        for k in range(KD):
            nc.tensor.matmul(lT[:], lhsT=wg_sb[:, k, :], rhs=xs[:, k, :],
                             start=(k == 0), stop=(k == KD - 1))
        lt_sb = rpool.tile([E, TOK_PER_CORE], f32, name="lt_sb")
        nc.vector.tensor_copy(lt_sb[:], lT[:])
        nc.sync.dma_start(l_dram[:], lt_sb[:])
        nc.gpsimd.collective_compute(
            "AllGather", mybir.AluOpType.bypass,
            replica_groups=[list(range(N_CORES))],
            ins=[l_dram[:].opt()], outs=[lg_dram[:].opt()],
        )
        # lg[(c e), tok_local]; token t*P+p has t = c*4 + t4,
        # tok_local = t4*P + p. Transpose each 128-token block on the PE.
        lg_sb = rpool.tile([N_CORES * E, TOK_PER_CORE], f32, name="lg_sb")
        nc.sync.dma_start(lg_sb[:], lg_dram[:])
        for k in range(KD):
            nc.sync.dma_start(w1_sb[k][:], w1d[k * P:(k + 1) * P, :])
        l_all = rpool.tile([P, NE], f32, name="l_all")
        l_all4 = l_all[:].rearrange("p (c t4 e) -> p c t4 e", c=N_CORES, t4=4)
        for t4 in range(4):
            ptp = rps.tile([P, N_CORES * E], f32, tag="ptp", bufs=2,
                           name="ptp")
            nc.tensor.transpose(ptp[:], lg_sb[:, t4 * P:(t4 + 1) * P],
                                identf[:N_CORES * E, :N_CORES * E])
            nc.vector.tensor_copy(
                l_all4[:, :, t4, :],
                ptp[:].rearrange("p (c e) -> p c e", e=E))
        nc.vector.tensor_add(l_all[:], l_all[:], bg_sb[:])
        if dbg is not None:
            nc.sync.dma_start(dbg["dbg_l"][:], l_all[:])
        nc.scalar.activation(exp_all[:], l_all[:],
                             mybir.ActivationFunctionType.Exp)
        # sums and reciprocal per token
        s_all = rpool.tile([P, NT], f32, name="s_all")
        nc.vector.reduce_sum(s_all[:], exp_all[:].rearrange(
            "p (t e) -> p t e", e=E), axis=mybir.AxisListType.X)
        nc.vector.reciprocal(r_all[:], s_all[:])
        # per-tile top8 + indices + top2 indicator
        for tt in range(NT):
            sl = slice(tt * E, (tt + 1) * E)
            nc.vector.max(out=m8_all[:, sl], in_=exp_all[:, sl])
            nc.vector.max_index(out=ei_all[:, sl], in_max=m8_all[:, sl],
                                in_values=exp_all[:, sl])
            nc.vector.tensor_scalar(
                ind_all[:, sl], exp_all[:, sl],
                m8_all[:, tt * E + 1:tt * E + 2], None,
                op0=mybir.AluOpType.is_ge)

    # ---------------- dispatch: positions + scatter slot->token map --------
    with tc.tile_pool(name="disp_sb", bufs=1, named_scope="dispatch") as dpool, \
         tc.tile_pool(name="disp_ps", bufs=1, space="PSUM") as dps:
        # per-tile totals: ptot[0, (t e)] = sum_p ind_all[p, (t e)]
        ptot = dps.tile([1, NE], f32, name="ptot")
        nc.tensor.matmul(ptot[:], lhsT=ones_col[:], rhs=ind_all[:],
                         start=True, stop=True)
        tot_flat = dpool.tile([1, NE], f32, name="tot_flat")
        nc.vector.tensor_copy(tot_flat[:], ptot[:])
        # reshape [1, NT*E] -> [NT, E] via sbuf-to-sbuf DMA
        tot32 = dpool.tile([NT, E], f32, name="tot32")
        nc.sync.dma_start(tot32[:], tot_flat[:])
        # exclusive cumsum over tiles: strict-lower matmul
        pofs = dps.tile([NT, E], f32, name="pofs")
        nc.tensor.matmul(pofs[:], lhsT=l128_sb[:NT, :NT], rhs=tot32[:],
                         start=True, stop=True)
        ofs32 = dpool.tile([NT, E], f32, name="ofs32")
        nc.vector.tensor_copy(ofs32[:], pofs[:])
        ofs_flat = dpool.tile([1, NE], f32, name="ofs_flat")
        nc.sync.dma_start(ofs_flat[:], ofs32[:])
        # positions: local excl cumsum (over partitions) + tile offset
        ppos = dps.tile([P, NE], f32, name="ppos")
        nc.tensor.matmul(ppos[:], lhsT=l128_sb[:], rhs=ind_all[:],
                         start=True, stop=False)
        nc.tensor.matmul(ppos[:], lhsT=ones_row[:], rhs=ofs_flat[:],
                         start=False, stop=True)
        nc.vector.tensor_copy(pos_all[:], ppos[:])

        # my expert's positions / indicator
        tmp = dpool.tile([P, NE], f32, name="tmp")
        nc.vector.tensor_mul(tmp[:], pos_all[:], sel_sb[:])
        pos_e = dpool.tile([P, NT], f32, name="pos_e")
        nc.vector.reduce_sum(pos_e[:], tmp[:].rearrange(
            "p (t e) -> p t e", e=E), axis=mybir.AxisListType.X)
        nc.vector.tensor_mul(tmp[:], ind_all[:], sel_sb[:])
        ind_e = dpool.tile([P, NT], f32, name="ind_e")
        nc.vector.reduce_sum(ind_e[:], tmp[:].rearrange(
            "p (t e) -> p t e", e=E), axis=mybir.AxisListType.X)
        # masked positions: ind ? pos : BIGPOS (dropped by bounds check)
        pos_m = dpool.tile([P, NT], f32, name="pos_m")
        nc.vector.tensor_scalar_add(pos_m[:], pos_e[:], -BIGPOS)
        nc.vector.tensor_mul(pos_m[:], pos_m[:], ind_e[:])
        nc.vector.tensor_scalar_add(pos_m[:], pos_m[:], BIGPOS)
        pos_i = dpool.tile([P, NT], i32, name="pos_i")
        nc.vector.tensor_copy(pos_i[:], pos_m[:])
        # remapped scatter target: pos2 = (pos & 127)*ST + (pos >> 7), so the
        # readback [C] -> [P, ST] is a contiguous per-partition DMA.
        # BIGPOS remaps to 8192 >= C and is still dropped by the bounds check.
        ph1 = dpool.tile([P, NT], i32, name="ph1")
        nc.vector.tensor_scalar(ph1[:], pos_i[:], 7, None,
                                op0=mybir.AluOpType.logical_shift_right)
        ph2 = dpool.tile([P, NT], i32, name="ph2")
        nc.vector.tensor_scalar(ph2[:], pos_i[:], 127, None,
                                op0=mybir.AluOpType.bitwise_and)
        nc.vector.tensor_scalar(ph2[:], ph2[:], ST, None,
                                op0=mybir.AluOpType.mult)
        pos2 = dpool.tile([P, NT], i32, name="pos2")
        nc.vector.tensor_add(pos2[:], ph1[:], ph2[:])

        # token ids: tokid[p, t] = t*P + p
        tokid = dpool.tile([P, NT], i32, name="tokid")
        nc.gpsimd.iota(tokid[:], pattern=[[P, NT]], base=0,
                       channel_multiplier=1)
        # zero-init the partial maps (padding slots gather token 0)
        zsb = dpool.tile([1, C], i32, name="zsb")
        nc.vector.memset(zsb[:], 0)
        for k in range(8):
            nc.sync.dma_start(
                smaps[k][:].rearrange("(a c) o -> a (c o)", a=1), zsb[:])
        # HW SWDGE indirect DMA processes one offset per partition, so
        # scatter one 128-token column at a time; round-robin over the 8
        # partial maps so the WAW chains are only 4 deep.
        for t in range(NT):
            nc.gpsimd.indirect_dma_start(
                out=smaps[t % 8][:],
                out_offset=IndirectOffsetOnAxis(ap=pos2[:, t:t + 1], axis=0),
                in_=tokid[:, t:t + 1], in_offset=None,
                bounds_check=C - 1, oob_is_err=False,
            )
        # read back (contiguous per partition) and merge: slot s*P+p sits at
        # remapped index p*ST+s, i.e. idx_sb[p, s] after the reshape
        idxp = [dpool.tile([P, ST], i32, name=f"idxp{k}") for k in range(8)]
        for k in range(8):
            nc.sync.dma_start(
                idxp[k][:], smaps[k][:].rearrange("(p s) o -> p (s o)", p=P))
        for k in (0, 2, 4, 6):
            nc.vector.tensor_add(idxp[k][:], idxp[k][:], idxp[k + 1][:])
        nc.vector.tensor_add(idxp[0][:], idxp[0][:], idxp[2][:])
        nc.vector.tensor_add(idxp[4][:], idxp[4][:], idxp[6][:])
        nc.vector.tensor_add(idx_sb[:], idxp[0][:], idxp[4][:])
        if dbg is not None:
            nc.sync.dma_start(dbg["dbg_pos"][:], pos_all[:])
            nc.sync.dma_start(dbg["dbg_posm"][:], pos_m[:])
            nc.sync.dma_start(dbg["dbg_idx"][:], idx_sb[:])

        # gather tokens (bf16) and transpose into xgT
        with tc.tile_pool(name="gat_sb", bufs=2) as gpool, \
             tc.tile_pool(name="gat_ps", bufs=2, space="PSUM") as gps:
            for s in range(ST):
                xg = gpool.tile([P, D], bf16, tag="xg", name="xg")
                nc.gpsimd.indirect_dma_start(
                    out=xg[:], out_offset=None, in_=xbf[:],
                    in_offset=IndirectOffsetOnAxis(ap=idx_sb[:, s:s + 1],
                                                   axis=0),
                )
                for d in range(KD):
                    pt = gps.tile([P, P], bf16, tag="pt", name="pt")
                    nc.tensor.transpose(pt[:], xg[:, d * P:(d + 1) * P],
                                        ident[:])
                    nc.vector.tensor_copy(xgT[d][:, s * P:(s + 1) * P], pt[:])

    # ---------------- combine planes (needs only router/dispatch state) ----
    cpool = tc.alloc_tile_pool(name="comb_sb", bufs=1)
    with tc.tile_pool(name="comb_tmp", bufs=2, named_scope="combine") as ctmp:
        # expert ids of top-1/top-2 as f32
        e1f = ctmp.tile([P, NT], f32, tag="e1f", bufs=1, name="e1f")
        e2f = ctmp.tile([P, NT], f32, tag="e2f", bufs=1, name="e2f")
        ei3 = ei_all[:].rearrange("p (t e) -> p t e", e=E)
        nc.vector.tensor_copy(e1f[:], ei3[:, :, 0])
        nc.vector.tensor_copy(e2f[:], ei3[:, :, 1])
        ioz = ctmp.tile([P, NE], i32, tag="ioz", bufs=1, name="ioz")
        nc.gpsimd.iota(ioz[:].rearrange("p (t e) -> p t e", e=E),
                       pattern=[[0, NT], [1, E]], base=0, channel_multiplier=0)
        iof = ctmp.tile([P, NE], f32, tag="iof", bufs=1, name="iof")
        nc.vector.tensor_copy(iof[:], ioz[:])
        m83 = m8_all[:].rearrange("p (t e) -> p t e", e=E)

        # Selection stack S: for each (q, g) a row-index plane into y_all[g]
        # (rows not in chunk g point at row 0) and a masked gate-weight
        # plane; one masked reduce per owner tile pulls all 12 values.
        NSEL = 2 * NG
        S = ctmp.tile([P, 2 * NSEL, NT], f32, bufs=1, name="S")
        for q, ef in ((0, e1f), (1, e2f)):
            oh = ctmp.tile([P, NE], f32, tag=f"oh{q}", bufs=1, name=f"oh{q}")
            nc.vector.tensor_tensor(
                out=oh[:].rearrange("p (t e) -> p t e", e=E),
                in0=iof[:].rearrange("p (t e) -> p t e", e=E),
                in1=ef[:, :, None].to_broadcast([P, NT, E]),
                op=mybir.AluOpType.is_equal)
            nc.vector.tensor_mul(oh[:], oh[:], pos_all[:])
            slot = ctmp.tile([P, NT], f32, tag=f"slot{q}", bufs=1,
                             name=f"slot{q}")
            nc.vector.reduce_sum(slot[:], oh[:].rearrange(
                "p (t e) -> p t e", e=E), axis=mybir.AxisListType.X)
            # chunk id g = (slot>=CHK) + (slot>=2*CHK) + ...
            gch = ctmp.tile([P, NT], f32, tag=f"gch{q}", bufs=1,
                            name=f"gch{q}")
            nc.vector.tensor_scalar(gch[:], slot[:], float(CHK), None,
                                    op0=mybir.AluOpType.is_ge)
            for gg in range(2, NG):
                t2 = ctmp.tile([P, NT], f32, tag="t2", name="t2")
                nc.vector.tensor_scalar(t2[:], slot[:], float(CHK * gg), None,
                                        op0=mybir.AluOpType.is_ge)
                nc.vector.tensor_add(gch[:], gch[:], t2[:])
            # in-chunk row: e*CHK + (slot - g*CHK)
            base = ctmp.tile([P, NT], f32, tag=f"base{q}", bufs=1,
                             name=f"base{q}")
            nc.vector.scalar_tensor_tensor(
                out=base[:], in0=ef[:], scalar=float(CHK), in1=slot[:],
                op0=mybir.AluOpType.mult, op1=mybir.AluOpType.add)
            gv = ctmp.tile([P, NT], f32, tag=f"gv{q}", bufs=1, name=f"gv{q}")
            nc.vector.tensor_tensor(out=gv[:], in0=m83[:, :, q], in1=r_all[:],
                                    op=mybir.AluOpType.mult)
            for gg in range(NG):
                k = q * NG + gg
                eq = ctmp.tile([P, NT], f32, tag="eq", name="eq")
                nc.vector.tensor_scalar(eq[:], gch[:], float(gg), None,
                                        op0=mybir.AluOpType.is_equal)
                nc.vector.tensor_scalar_add(S[:, k, :], base[:],
                                            float(-CHK * gg))
                nc.vector.tensor_mul(S[:, k, :], S[:, k, :], eq[:])
                nc.vector.tensor_mul(S[:, NSEL + k, :], eq[:], gv[:])

        if dbg is not None:
            nc.sync.dma_start(
                dbg["dbg_S"][:].rearrange("p (k t) -> p k t", k=2 * NSEL),
                S[:])
        # per-owner-tile row indices and weights, ready before the FFN
        reds = []
        redis = []
        for j in range(OWN_TILES):
            own = ownm_sb[:, j * NT:(j + 1) * NT]
            tmpS = ctmp.tile([P, 2 * NSEL, NT], f32, tag="tmpS", bufs=2,
                             name="tmpS")
            nc.vector.tensor_tensor(
                out=tmpS[:], in0=S[:],
                in1=own[:, None, :].to_broadcast([P, 2 * NSEL, NT]),
                op=mybir.AluOpType.mult)
            red = cpool.tile([P, 2 * NSEL], f32, name=f"red{j}")
            nc.vector.reduce_sum(red[:], tmpS[:], axis=mybir.AxisListType.X)
            redi = cpool.tile([P, NSEL], i32, name=f"redi{j}")
            nc.vector.tensor_copy(redi[:], red[:, :NSEL])
            reds.append(red)
            redis.append(redi)
    ots = [cpool.tile([P, D], f32, name=f"ot{j}") for j in range(OWN_TILES)]

    # ---------------- FFN (bf16) + chunked y AllGather + combine ----------
    with tc.tile_pool(name="ffn_sb", bufs=1, named_scope="ffn") as fpool, \
         tc.tile_pool(name="ffn_ps", bufs=1, space="PSUM") as fps:
        for g in range(NG):
            t0 = g * TG
            py = [[fps.tile([P, 512], f32, tag=f"py_{t}_{n}",
                            name=f"py_{t}_{n}")
                   for n in range(2)] for t in range(TG)]
            for f in range(NF):
                ph = fps.tile([P, CHK], f32, tag="ph", bufs=2, name="ph")
                for k in range(KD):
                    nc.tensor.matmul(
                        ph[:], lhsT=w1_sb[k][:, f * P:(f + 1) * P],
                        rhs=xgT[k][:, t0 * P:t0 * P + CHK],
                        start=(k == 0), stop=(k == KD - 1))
                hbuf = fpool.tile([P, CHK], bf16, tag="hbuf", bufs=3,
                                  name="hbuf")
                nc.scalar.activation(hbuf[:], ph[:],
                                     mybir.ActivationFunctionType.Relu,
                                     bias=b1_sb[:, f:f + 1], scale=1.0)
                w2f = fpool.tile([P, D], bf16, tag="w2f", bufs=3, name="w2f")
                nc.sync.dma_start(w2f[:], w2d[f * P:(f + 1) * P, :])
                for t in range(TG):
                    for n in range(2):
                        nc.tensor.matmul(
                            py[t][n][:],
                            lhsT=hbuf[:, t * P:(t + 1) * P],
                            rhs=w2f[:, n * 512:(n + 1) * 512],
                            start=(f == 0), stop=(f == NF - 1))
            # add b2 (replicated across partitions) during PSUM drain
            for t in range(TG):
                ysb = fpool.tile([P, D], bf16, tag="ysb", bufs=2, name="ysb")
                for n in range(2):
                    nc.vector.tensor_tensor(
                        out=ysb[:, n * 512:(n + 1) * 512], in0=py[t][n][:],
                        in1=b2_sb[:, n * 512:(n + 1) * 512],
                        op=mybir.AluOpType.add)
                nc.sync.dma_start(y_dram[g][t * P:(t + 1) * P, :], ysb[:])
            # ship this chunk while the next group computes
            nc.gpsimd.collective_compute(
                "AllGather", mybir.AluOpType.bypass,
                replica_groups=[list(range(N_CORES))],
                ins=[y_dram[g][:].opt()],
                outs=[y_all[g][:].opt()],
            )
            # combine this chunk's contributions while later groups compute
            with tc.tile_pool(name=f"comb_g{g}", bufs=2,
                              named_scope="combine") as cg:
                for j in range(OWN_TILES):
                    for q in range(2):
                        k = q * NG + g
                        yt = cg.tile([P, D], bf16, tag=f"yt{j}_{q}",
                                     name=f"yt{j}_{q}")
                        nc.gpsimd.indirect_dma_start(
                            out=yt[:], out_offset=None, in_=y_all[g][:],
                            in_offset=IndirectOffsetOnAxis(
                                ap=redis[j][:, k:k + 1], axis=0))
                        w = reds[j][:, NSEL + k:NSEL + k + 1]
                        if g == 0 and q == 0:
                            nc.vector.tensor_scalar(
                                ots[j][:], yt[:], w, None,
                                op0=mybir.AluOpType.mult)
                        else:
                            nc.vector.scalar_tensor_tensor(
                                out=ots[j][:], in0=yt[:], scalar=w,
                                in1=ots[j][:], op0=mybir.AluOpType.mult,
                                op1=mybir.AluOpType.add)
        if dbg is not None:
            nc.sync.dma_start(dbg["dbg_xgT"][:], xgT[0][:])
            for g in range(NG):
                nc.sync.dma_start(dbg["dbg_y"][g * CHK:(g + 1) * CHK, :],
                                  y_dram[g][:])

    for j in range(OWN_TILES):
        nc.sync.dma_start(out[j * P:(j + 1) * P, :], ots[j][:])

    cpool.release()
    persist.release()
    dram.release()


def _host_prep(x, Wg, bg, W1, b1, W2, b2, C):
    xf = np.ascontiguousarray(x.reshape(T, D).astype(np.float32))
    xT = np.ascontiguousarray(xf.T)
    xbf = xf.astype(ml_dtypes.bfloat16)
    bgb = np.tile(bg.astype(np.float32), NT)[None, :].repeat(P, 0)
    bgb = np.ascontiguousarray(bgb)
    l128 = np.triu(np.ones((P, P), np.float32), 1)  # [t', t] = 1 if t' < t
    in_maps = []
    for c in range(N_CORES):
        sel = np.zeros(E, np.float32)
        sel[c] = 1.0
        sel256 = np.ascontiguousarray(np.tile(sel, NT)[None, :].repeat(P, 0))
        ownm = np.zeros((P, OWN_TILES, NT), np.float32)
        for j in range(OWN_TILES):
            ownm[:, j, OWN_TILES * c + j] = 1.0
        in_maps.append({
            "xTs": np.ascontiguousarray(
                xT[:, c * TOK_PER_CORE:(c + 1) * TOK_PER_CORE]),
            "xbf": xbf,
            "w1d": np.ascontiguousarray(W1[c].astype(ml_dtypes.bfloat16)),
            "w2d": np.ascontiguousarray(W2[c].astype(ml_dtypes.bfloat16)),
            "wgd": np.ascontiguousarray(Wg.astype(np.float32)),
            "bgb": bgb,
            "b1pm": np.ascontiguousarray(
                b1[c].astype(np.float32).reshape(NF, P).T),
            "b2pm": np.ascontiguousarray(
                np.tile(b2[c].astype(np.float32)[None, :], (P, 1))),
            "sel256": sel256,
            "l128d": l128,
            "ownmd": np.ascontiguousarray(ownm.reshape(P, OWN_TILES * NT)),
        })
    return in_maps


def _capacity(x, Wg, bg):
    xf = x.reshape(T, D).astype(np.float32)
    logits = xf @ Wg.astype(np.float32) + bg.astype(np.float32)
    part = np.partition(logits, E - 2, axis=-1)
    m2 = part[:, E - 2:E - 1]
    counts = (logits >= m2).sum(0)
    return int(np.ceil((counts.max() + 16) / CHK) * CHK)


LAST_RESULT = None


def kernel(x, Wg, bg, W1, b1, W2, b2):
    global LAST_RESULT
    from concourse.bass_utils import run_bass_kernel_spmd

    x = np.asarray(x)
    C = _capacity(x, np.asarray(Wg), np.asarray(bg))
    if C not in _cache:
        _cache[C] = build_module(C)
    nc = _cache[C]
    in_maps = _host_prep(x, np.asarray(Wg), np.asarray(bg), np.asarray(W1),
                         np.asarray(b1), np.asarray(W2), np.asarray(b2), C)
    trace = bool(os.environ.get("BASS_TRACE"))
    if trace:
        _setup_axon_profile_hook()
    res = run_bass_kernel_spmd(nc, in_maps, core_ids=list(range(N_CORES)),
                               trace=trace)
    LAST_RESULT = res
    out = np.empty((T, D), np.float32)
    for c in range(N_CORES):
        out[c * TOK_PER_CORE:(c + 1) * TOK_PER_CORE] = res.results[c]["out"]
    return out.reshape(B, S, D)


def _setup_axon_profile_hook():
    """Provide antenv.axon_hooks (missing in this image) so trace=True works."""
    import types
    try:
        import antenv
        if "antenv.axon_hooks" not in sys.modules:
            hooks = types.ModuleType("antenv.axon_hooks")
            hooks._hook = None
            hooks.set_axon_ntff_profile_hook = \
                lambda h: setattr(hooks, "_hook", h)
            hooks.get_axon_ntff_profile_hook = lambda: hooks._hook
            sys.modules["antenv.axon_hooks"] = hooks
            antenv.axon_hooks = hooks
            from trn_agent_boot.trn_boot import _ntff_profile_via_ctypes
            hooks.set_axon_ntff_profile_hook(
                _ntff_profile_via_ctypes("/opt/axon/libaxon_pjrt.so"))
    except Exception as e:  # profiling is best-effort
        print(f"profile hook setup failed: {e}", file=sys.stderr)


# revision 34
# speedup vs baseline: 1.0411x; 1.0411x over previous
"""Expert-parallel top-2 MoE kernel for 8 Trainium2 NeuronCores.

Strategy (expert-parallel, sparse dispatch, per the sharding hint):
  - Router sharded over cores: core c computes fp32 logits for its 512-token
    shard on the TensorEngine (Wg stationary), AllGathers them (contiguous
    [E, tok] layout, transposed back on the PE) so every core holds identical
    logits for all 4096 tokens; softmax/top-2 on-device.
  - Core c owns expert c: slot positions come from matmul-based exclusive
    cumsums; the slot->token map is built with a single indirect-scatter DMA
    (token ids scattered to their slot positions; unrouted tokens get an
    out-of-bounds position and are dropped via bounds_check). Routed tokens
    are gathered via indirect DMA, transposed on the TensorEngine, and run
    through the two-layer FFN in bf16 (capacity padded to a multiple of 384).
  - Unscaled expert outputs (bf16) are AllGathered chunk-by-chunk into one
    shared [NG*8*CHK, D] tensor (overlapped with the FFN); each core combines
    the top-2 contributions for its own 512-token shard with two indirect
    row-gathers per 128-token tile + gate-weighted sum in fp32.

Numerics: router fp32 (top-2 selection fidelity), FFN bf16 with fp32
accumulation in PSUM, combine in fp32.
"""

import os
import sys

import numpy as np

for _p in ("/opt/trn_rl_repo",):
    if _p not in sys.path:
        sys.path.append(_p)

import ml_dtypes

import concourse.bass as bass
import concourse.mybir as mybir
import concourse.tile as tile
from concourse import bacc
from concourse.bass import IndirectOffsetOnAxis
from concourse.masks import make_identity

# Problem shapes (fixed per spec)
B, S, D, E = 2, 2048, 1024, 8
T = B * S          # 4096 tokens
F = 4 * D          # 4096 ffn dim
P = 128            # partitions
NT = T // P        # 32 token tiles
KD = D // P        # 8 contraction tiles over D
NF = F // P        # 32 f tiles
TOK_PER_CORE = T // E   # 512
OWN_TILES = TOK_PER_CORE // P  # 4
N_CORES = E
TG = 3                       # slot tiles per FFN group
CHK = TG * P                 # 384: slot chunk for the chunked AllGather
BIGPOS = float(1 << 20)      # scatter position for unrouted tokens (dropped)

f32 = mybir.dt.float32
bf16 = mybir.dt.bfloat16
f16 = mybir.dt.float16
i32 = mybir.dt.int32
u32 = mybir.dt.uint32

_cache = {}


def build_module(C: int, debug_out: bool = False):
    """Build the SPMD Bass module for capacity C (multiple of 384)."""
    assert C % CHK == 0
    ST = C // P  # slot tiles per expert

    nc = bacc.Bacc("TRN2", target_bir_lowering=False, debug=False,
                   num_devices=N_CORES)

    # ---- I/O ----
    xTs = nc.dram_tensor("xTs", [D, TOK_PER_CORE], f32,
                         kind="ExternalInput").ap()
    xbf = nc.dram_tensor("xbf", [T, D], bf16, kind="ExternalInput").ap()
    w1d = nc.dram_tensor("w1d", [D, F], bf16, kind="ExternalInput").ap()
    w2d = nc.dram_tensor("w2d", [F, D], bf16, kind="ExternalInput").ap()
    wgd = nc.dram_tensor("wgd", [D, E], f32, kind="ExternalInput").ap()
    bgb = nc.dram_tensor("bgb", [P, NT * E], f32, kind="ExternalInput").ap()
    b1pm = nc.dram_tensor("b1pm", [P, NF], f32, kind="ExternalInput").ap()
    b2pm = nc.dram_tensor("b2pm", [P, D], f32, kind="ExternalInput").ap()
    sel256 = nc.dram_tensor("sel256", [P, NT * E], f32,
                            kind="ExternalInput").ap()
    l128d = nc.dram_tensor("l128d", [P, P], f32, kind="ExternalInput").ap()
    ownmd = nc.dram_tensor("ownmd", [P, OWN_TILES * NT], f32,
                           kind="ExternalInput").ap()
    out = nc.dram_tensor("out", [TOK_PER_CORE, D], f32,
                         kind="ExternalOutput").ap()
    dbg = None
    if debug_out:
        dbg = {
            "dbg_l": nc.dram_tensor("dbg_l", [P, NT * E], f32,
                                    kind="ExternalOutput").ap(),
            "dbg_pos": nc.dram_tensor("dbg_pos", [P, NT * E], f32,
                                      kind="ExternalOutput").ap(),
            "dbg_posm": nc.dram_tensor("dbg_posm", [P, NT], f32,
                                       kind="ExternalOutput").ap(),
            "dbg_idx": nc.dram_tensor("dbg_idx", [P, C // P], i32,
                                      kind="ExternalOutput").ap(),
            "dbg_xgT": nc.dram_tensor("dbg_xgT", [P, C], bf16,
                                      kind="ExternalOutput").ap(),
            "dbg_y": nc.dram_tensor("dbg_y", [C, D], bf16,
                                    kind="ExternalOutput").ap(),
            "dbg_S": nc.dram_tensor("dbg_S", [P, 4 * (C // CHK) * NT], f32,
                                    kind="ExternalOutput").ap(),
        }

    with tile.TileContext(nc) as tc:
        _emit(tc, C, ST, xTs, xbf, w1d, w2d, wgd, bgb, b1pm, b2pm, sel256,
              l128d, ownmd, out, dbg)

    nc.compile()
    return nc


def _emit(tc, C, ST, xTs, xbf, w1d, w2d, wgd, bgb, b1pm, b2pm, sel256,
          l128d, ownmd, out, dbg=None):
    nc = tc.nc
    NE = NT * E  # 256
    NG = ST // TG  # number of FFN groups / y-AllGather chunks
    GBLK = N_CORES * CHK  # rows per AllGather chunk in y_big

    # ---------------- persistent pools ----------------
    persist = tc.alloc_tile_pool(name="persist", bufs=1)
    dram = tc.alloc_tile_pool(name="dram", bufs=1, space="DRAM")

    # tiny warmup AllGather: absorbs first-collective setup cost and aligns
    # the cores before the logits AllGather on the critical path
    wup_in = dram.tile([E, 4], f32, name="wup_in")
    wup_out = dram.tile([N_CORES * E, 4], f32, addr_space="Shared",
                        name="wup_out")
    nc.gpsimd.collective_compute(
        "AllGather", mybir.AluOpType.bypass,
        replica_groups=[list(range(N_CORES))],
        ins=[wup_in[:].opt()], outs=[wup_out[:].opt()],
    )

    # constants / weights resident in SBUF
    wg_sb = persist.tile([P, KD, E], f32, name="wg_sb")
    nc.sync.dma_start(wg_sb[:], wgd.rearrange("(k p) e -> p k e", p=P))
    bg_sb = persist.tile([P, NE], f32, name="bg_sb")
    nc.sync.dma_start(bg_sb[:], bgb[:])
    sel_sb = persist.tile([P, NE], f32, name="sel_sb")
    nc.sync.dma_start(sel_sb[:], sel256[:])
    l128_sb = persist.tile([P, P], f32, name="l128_sb")
    nc.sync.dma_start(l128_sb[:], l128d[:])
    ownm_sb = persist.tile([P, OWN_TILES * NT], f32, name="ownm_sb")
    nc.sync.dma_start(ownm_sb[:], ownmd[:])
    b1_sb = persist.tile([P, NF], f32, name="b1_sb")
    nc.sync.dma_start(b1_sb[:], b1pm[:])
    b2_sb = persist.tile([P, D], f32, name="b2_sb")
    nc.sync.dma_start(b2_sb[:], b2pm[:])
    ident = persist.tile([P, P], bf16, name="ident")
    make_identity(nc, ident[:])
    identf = persist.tile([P, P], f32, name="identf")
    make_identity(nc, identf[:])
    ones_col = persist.tile([P, 1], f32, name="ones_col")
    nc.vector.memset(ones_col[:], 1.0)
    ones_row = persist.tile([1, P], f32, name="ones_row")
    nc.vector.memset(ones_row[:], 1.0)

    # w1 tiles allocated here; the 8MB of loads are issued after the router's
    # small DMAs so they don't head-of-line block the critical path
    w1_sb = [persist.tile([P, F], bf16, name=f"w1_sb{k}") for k in range(KD)]

    # router / dispatch state kept for the combine phase
    exp_all = persist.tile([P, NE], f32, name="exp_all")    # exp(logits)
    m8_all = persist.tile([P, NE], f32, name="m8_all")      # per-tile top8 of exp
    r_all = persist.tile([P, NT], f32, name="r_all")        # 1/sum(exp)
    pos_all = persist.tile([P, NE], f32, name="pos_all")    # excl cumsum per expert
    ind_all = persist.tile([P, NE], f32, name="ind_all")    # top2 indicator
    ei_all = persist.tile([P, NE], u32, name="ei_all")      # top8 expert indices
    idx_sb = persist.tile([P, ST], i32, name="idx_sb")      # slot -> token id

    xgT = [persist.tile([P, C], bf16, name=f"xgT{d}") for d in range(KD)]

    l_dram = dram.tile([E, TOK_PER_CORE], f32, name="l_dram")
    lg_dram = dram.tile([N_CORES * E, TOK_PER_CORE], f32, addr_space="Shared",
                        name="lg_dram")
    y_dram = [dram.tile([CHK, D], bf16, name=f"y_dram{g}") for g in range(NG)]
    # one Shared AllGather output per chunk (Shared tensors allow one writer)
    y_all = [dram.tile([N_CORES * CHK, D], bf16, addr_space="Shared",
                       name=f"y_all{g}") for g in range(NG)]

    # ---------------- router (sharded + AllGather) ----------------
    with tc.tile_pool(name="router_sb", bufs=1, named_scope="router") as rpool, \
         tc.tile_pool(name="router_ps", bufs=1, space="PSUM") as rps:
        xs = rpool.tile([P, KD, TOK_PER_CORE], f32, name="xs")
        nc.sync.dma_start(xs[:], xTs.rearrange("(k p) t -> p k t", p=P))
        lT = rps.tile([E, TOK_PER_CORE], f32, name="lT")
        for k in range(KD):
            nc.tensor.matmul(lT[:], lhsT=wg_sb[:, k, :], rhs=xs[:, k, :],
                             start=(k == 0), stop=(k == KD - 1))
        lt_sb = rpool.tile([E, TOK_PER_CORE], f32, name="lt_sb")
        nc.vector.tensor_copy(lt_sb[:], lT[:])
        nc.sync.dma_start(l_dram[:], lt_sb[:])
        nc.gpsimd.collective_compute(
            "AllGather", mybir.AluOpType.bypass,
            replica_groups=[list(range(N_CORES))],
            ins=[l_dram[:].opt()], outs=[lg_dram[:].opt()],
        )
        # lg[(c e), tok_local]; token t*P+p has t = c*4 + t4,
        # tok_local = t4*P + p. Transpose each 128-token block on the PE.
        lg_sb = rpool.tile([N_CORES * E, TOK_PER_CORE], f32, name="lg_sb")
        nc.sync.dma_start(lg_sb[:], lg_dram[:])
        for k in range(KD):
            nc.sync.dma_start(w1_sb[k][:], w1d[k * P:(k + 1) * P, :])
        l_all = rpool.tile([P, NE], f32, name="l_all")
        l_all4 = l_all[:].rearrange("p (c t4 e) -> p c t4 e", c=N_CORES, t4=4)
        for t4 in range(4):
            ptp = rps.tile([P, N_CORES * E], f32, tag="ptp", bufs=2,
                           name="ptp")
            nc.tensor.transpose(ptp[:], lg_sb[:, t4 * P:(t4 + 1) * P],
                                identf[:N_CORES * E, :N_CORES * E])
            nc.vector.tensor_copy(
                l_all4[:, :, t4, :],
                ptp[:].rearrange("p (c e) -> p c e", e=E))
        nc.vector.tensor_add(l_all[:], l_all[:], bg_sb[:])
        if dbg is not None:
            nc.sync.dma_start(dbg["dbg_l"][:], l_all[:])
        nc.scalar.activation(exp_all[:], l_all[:],
                             mybir.ActivationFunctionType.Exp)
        # sums and reciprocal per token
        s_all = rpool.tile([P, NT], f32, name="s_all")
        nc.vector.reduce_sum(s_all[:], exp_all[:].rearrange(
            "p (t e) -> p t e", e=E), axis=mybir.AxisListType.X)
        nc.vector.reciprocal(r_all[:], s_all[:])
        # per-tile top8 + top2 indicator (indices for the combine are
        # extracted later, off the dispatch critical path)
        for tt in range(NT):
            sl = slice(tt * E, (tt + 1) * E)
            nc.vector.max(out=m8_all[:, sl], in_=exp_all[:, sl])
            nc.vector.tensor_scalar(
                ind_all[:, sl], exp_all[:, sl],
                m8_all[:, tt * E + 1:tt * E + 2], None,
                op0=mybir.AluOpType.is_ge)

    # ---------------- dispatch: positions + scatter slot->token map --------
    with tc.tile_pool(name="disp_sb", bufs=1, named_scope="dispatch") as dpool, \
         tc.tile_pool(name="disp_ps", bufs=1, space="PSUM") as dps:
        # per-tile totals: ptot[0, (t e)] = sum_p ind_all[p, (t e)]
        ptot = dps.tile([1, NE], f32, name="ptot")
        nc.tensor.matmul(ptot[:], lhsT=ones_col[:], rhs=ind_all[:],
                         start=True, stop=True)
        tot_flat = dpool.tile([1, NE], f32, name="tot_flat")
        nc.vector.tensor_copy(tot_flat[:], ptot[:])
        # reshape [1, NT*E] -> [NT, E] via sbuf-to-sbuf DMA
        tot32 = dpool.tile([NT, E], f32, name="tot32")
        nc.sync.dma_start(tot32[:], tot_flat[:])
        # exclusive cumsum over tiles: strict-lower matmul
        pofs = dps.tile([NT, E], f32, name="pofs")
        nc.tensor.matmul(pofs[:], lhsT=l128_sb[:NT, :NT], rhs=tot32[:],
                         start=True, stop=True)
        ofs32 = dpool.tile([NT, E], f32, name="ofs32")
        nc.vector.tensor_copy(ofs32[:], pofs[:])
        ofs_flat = dpool.tile([1, NE], f32, name="ofs_flat")
        nc.sync.dma_start(ofs_flat[:], ofs32[:])
        # positions: local excl cumsum (over partitions) + tile offset
        ppos = dps.tile([P, NE], f32, name="ppos")
        nc.tensor.matmul(ppos[:], lhsT=l128_sb[:], rhs=ind_all[:],
                         start=True, stop=False)
        nc.tensor.matmul(ppos[:], lhsT=ones_row[:], rhs=ofs_flat[:],
                         start=False, stop=True)
        nc.vector.tensor_copy(pos_all[:], ppos[:])

        # my expert's positions / indicator
        tmp = dpool.tile([P, NE], f32, name="tmp")
        nc.vector.tensor_mul(tmp[:], pos_all[:], sel_sb[:])
        pos_e = dpool.tile([P, NT], f32, name="pos_e")
        nc.vector.reduce_sum(pos_e[:], tmp[:].rearrange(
            "p (t e) -> p t e", e=E), axis=mybir.AxisListType.X)
        nc.vector.tensor_mul(tmp[:], ind_all[:], sel_sb[:])
        ind_e = dpool.tile([P, NT], f32, name="ind_e")
        nc.vector.reduce_sum(ind_e[:], tmp[:].rearrange(
            "p (t e) -> p t e", e=E), axis=mybir.AxisListType.X)
        # masked positions: ind ? pos : BIGPOS (matches no slot)
        pos_m = dpool.tile([P, NT], f32, name="pos_m")
        nc.vector.tensor_scalar_add(pos_m[:], pos_e[:], -BIGPOS)
        nc.vector.tensor_mul(pos_m[:], pos_m[:], ind_e[:])
        nc.vector.tensor_scalar_add(pos_m[:], pos_m[:], BIGPOS)
        # remapped position pos2 = (pos & 127)*ST + (pos >> 7), so the final
        # departition DMA is contiguous per partition. BIGPOS remaps to
        # 8192 >= C (fp16-exact) and matches no slot.
        pos_i = dpool.tile([P, NT], i32, name="pos_i")
        nc.vector.tensor_copy(pos_i[:], pos_m[:])
        ph1 = dpool.tile([P, NT], i32, name="ph1")
        nc.vector.tensor_scalar(ph1[:], pos_i[:], 7, None,
                                op0=mybir.AluOpType.logical_shift_right)
        ph2 = dpool.tile([P, NT], i32, name="ph2")
        nc.vector.tensor_scalar(ph2[:], pos_i[:], 127, None,
                                op0=mybir.AluOpType.bitwise_and)
        nc.vector.tensor_scalar(ph2[:], ph2[:], ST, None,
                                op0=mybir.AluOpType.mult)
        pos2 = dpool.tile([P, NT], i32, name="pos2")
        nc.vector.tensor_add(pos2[:], ph1[:], ph2[:])
        pos_mh = dpool.tile([P, NT], f32, name="pos_mh")
        nc.vector.tensor_copy(pos_mh[:], pos2[:])

        # slot->token map via fp16 one-hot + rank-2 matmul:
        #   Pt[p, c] = (c == pos2[p, tt]);  token id = p + 128*tt, so
        #   accumulate [p-part; 128*tt-part] over tiles with a 2-col lhsT.
        # (These iota/const tiles have no deps and get scheduled early.)
        iotaC_i = dpool.tile([P, C], i32, name="iotaC_i")
        nc.gpsimd.iota(iotaC_i[:], pattern=[[1, C]], base=0,
                       channel_multiplier=0)
        iotaC_h = dpool.tile([P, C], f16, name="iotaC_h")
        nc.vector.tensor_copy(iotaC_h[:], iotaC_i[:])
        tokp_i = dpool.tile([P, 1], i32, name="tokp_i")
        nc.gpsimd.iota(tokp_i[:], pattern=[[0, 1]], base=0,
                       channel_multiplier=1)
        tokt_i = dpool.tile([P, NT], i32, name="tokt_i")
        nc.gpsimd.iota(tokt_i[:], pattern=[[P, NT]], base=0,
                       channel_multiplier=0)
        tok2 = dpool.tile([P, NT, 2], f16, name="tok2")
        nc.vector.tensor_copy(tok2[:, :, 0],
                              tokp_i[:, 0:1].to_broadcast([P, NT]))
        nc.vector.tensor_copy(tok2[:, :, 1], tokt_i[:])

        NCH = (C + 511) // 512
        pid_ps = [dps.tile([2, min(512, C - ch * 512)], f32,
                           name=f"pid{ch}") for ch in range(NCH)]
        for tt in range(NT):
            Pt = dpool.tile([P, C], f16, tag="Pt", bufs=4, name="Pt")
            nc.vector.tensor_scalar(Pt[:], iotaC_h[:],
                                    pos_mh[:, tt:tt + 1], None,
                                    op0=mybir.AluOpType.is_equal)
            for ch in range(NCH):
                c0 = ch * 512
                c1 = min(c0 + 512, C)
                nc.tensor.matmul(pid_ps[ch][:], lhsT=tok2[:, tt, :],
                                 rhs=Pt[:, c0:c1],
                                 start=(tt == 0), stop=(tt == NT - 1))
        pid_sb = dpool.tile([2, C], f32, name="pid_sb")
        for ch in range(NCH):
            c0 = ch * 512
            c1 = min(c0 + 512, C)
            nc.vector.tensor_copy(pid_sb[:, c0:c1], pid_ps[ch][:])
        # departition [2, C] -> [P, 2, ST] (contiguous per partition), then
        # token id = p-part + 128*tt-part
        idx2f = dpool.tile([P, 2, ST], f32, name="idx2f")
        for r in range(2):
            nc.sync.dma_start(idx2f[:, r, :], pid_sb[r:r + 1, :])
        idxf = dpool.tile([P, ST], f32, name="idxf")
        nc.vector.tensor_add(idxf[:], idx2f[:, 0, :], idx2f[:, 1, :])
        nc.vector.tensor_copy(idx_sb[:], idxf[:])
        if dbg is not None:
            nc.sync.dma_start(dbg["dbg_pos"][:], pos_all[:])
            nc.sync.dma_start(dbg["dbg_posm"][:], pos_m[:])
            nc.sync.dma_start(dbg["dbg_idx"][:], idx_sb[:])

        # gather tokens (bf16) and transpose into xgT
        with tc.tile_pool(name="gat_sb", bufs=2) as gpool, \
             tc.tile_pool(name="gat_ps", bufs=2, space="PSUM") as gps:
            for s in range(ST):
                xg = gpool.tile([P, D], bf16, tag="xg", name="xg")
                nc.gpsimd.indirect_dma_start(
                    out=xg[:], out_offset=None, in_=xbf[:],
                    in_offset=IndirectOffsetOnAxis(ap=idx_sb[:, s:s + 1],
                                                   axis=0),
                )
                for d in range(KD):
                    pt = gps.tile([P, P], bf16, tag="pt", name="pt")
                    nc.tensor.transpose(pt[:], xg[:, d * P:(d + 1) * P],
                                        ident[:])
                    nc.vector.tensor_copy(xgT[d][:, s * P:(s + 1) * P], pt[:])

    # ---------------- combine planes (needs only router/dispatch state) ----
    cpool = tc.alloc_tile_pool(name="comb_sb", bufs=1)
    with tc.tile_pool(name="comb_tmp", bufs=2, named_scope="combine") as ctmp:
        # top-8 indices (only top-2 used), off the dispatch critical path
        for tt in range(NT):
            sl = slice(tt * E, (tt + 1) * E)
            nc.vector.max_index(out=ei_all[:, sl], in_max=m8_all[:, sl],
                                in_values=exp_all[:, sl])
        # expert ids of top-1/top-2 as f32
        e1f = ctmp.tile([P, NT], f32, tag="e1f", bufs=1, name="e1f")
        e2f = ctmp.tile([P, NT], f32, tag="e2f", bufs=1, name="e2f")
        ei3 = ei_all[:].rearrange("p (t e) -> p t e", e=E)
        nc.vector.tensor_copy(e1f[:], ei3[:, :, 0])
        nc.vector.tensor_copy(e2f[:], ei3[:, :, 1])
        ioz = ctmp.tile([P, NE], i32, tag="ioz", bufs=1, name="ioz")
        nc.gpsimd.iota(ioz[:].rearrange("p (t e) -> p t e", e=E),
                       pattern=[[0, NT], [1, E]], base=0, channel_multiplier=0)
        iof = ctmp.tile([P, NE], f32, tag="iof", bufs=1, name="iof")
        nc.vector.tensor_copy(iof[:], ioz[:])
        m83 = m8_all[:].rearrange("p (t e) -> p t e", e=E)

        # Selection stack S: for each (q, g) a row-index plane into y_all[g]
        # (rows not in chunk g point at row 0) and a masked gate-weight
        # plane; one masked reduce per owner tile pulls all 12 values.
        NSEL = 2 * NG
        S = ctmp.tile([P, 2 * NSEL, NT], f32, bufs=1, name="S")
        for q, ef in ((0, e1f), (1, e2f)):
            oh = ctmp.tile([P, NE], f32, tag=f"oh{q}", bufs=1, name=f"oh{q}")
            nc.vector.tensor_tensor(
                out=oh[:].rearrange("p (t e) -> p t e", e=E),
                in0=iof[:].rearrange("p (t e) -> p t e", e=E),
                in1=ef[:, :, None].to_broadcast([P, NT, E]),
                op=mybir.AluOpType.is_equal)
            nc.vector.tensor_mul(oh[:], oh[:], pos_all[:])
            slot = ctmp.tile([P, NT], f32, tag=f"slot{q}", bufs=1,
                             name=f"slot{q}")
            nc.vector.reduce_sum(slot[:], oh[:].rearrange(
                "p (t e) -> p t e", e=E), axis=mybir.AxisListType.X)
            # chunk id g = (slot>=CHK) + (slot>=2*CHK) + ...
            gch = ctmp.tile([P, NT], f32, tag=f"gch{q}", bufs=1,
                            name=f"gch{q}")
            nc.vector.tensor_scalar(gch[:], slot[:], float(CHK), None,
                                    op0=mybir.AluOpType.is_ge)
            for gg in range(2, NG):
                t2 = ctmp.tile([P, NT], f32, tag="t2", name="t2")
                nc.vector.tensor_scalar(t2[:], slot[:], float(CHK * gg), None,
                                        op0=mybir.AluOpType.is_ge)
                nc.vector.tensor_add(gch[:], gch[:], t2[:])
            # in-chunk row: e*CHK + (slot - g*CHK)
            base = ctmp.tile([P, NT], f32, tag=f"base{q}", bufs=1,
                             name=f"base{q}")
            nc.vector.scalar_tensor_tensor(
                out=base[:], in0=ef[:], scalar=float(CHK), in1=slot[:],
                op0=mybir.AluOpType.mult, op1=mybir.AluOpType.add)
            gv = ctmp.tile([P, NT], f32, tag=f"gv{q}", bufs=1, name=f"gv{q}")
            nc.vector.tensor_tensor(out=gv[:], in0=m83[:, :, q], in1=r_all[:],
                                    op=mybir.AluOpType.mult)
            for gg in range(NG):
                k = q * NG + gg
                eq = ctmp.tile([P, NT], f32, tag="eq", name="eq")
                nc.vector.tensor_scalar(eq[:], gch[:], float(gg), None,
                                        op0=mybir.AluOpType.is_equal)
                nc.vector.tensor_scalar_add(S[:, k, :], base[:],
                                            float(-CHK * gg))
                nc.vector.tensor_mul(S[:, k, :], S[:, k, :], eq[:])
                nc.vector.tensor_mul(S[:, NSEL + k, :], eq[:], gv[:])

        if dbg is not None:
            nc.sync.dma_start(
                dbg["dbg_S"][:].rearrange("p (k t) -> p k t", k=2 * NSEL),
                S[:])
        # per-owner-tile row indices and weights, ready before the FFN
        reds = []
        redis = []
        for j in range(OWN_TILES):
            own = ownm_sb[:, j * NT:(j + 1) * NT]
            tmpS = ctmp.tile([P, 2 * NSEL, NT], f32, tag="tmpS", bufs=2,
                             name="tmpS")
            nc.vector.tensor_tensor(
                out=tmpS[:], in0=S[:],
                in1=own[:, None, :].to_broadcast([P, 2 * NSEL, NT]),
                op=mybir.AluOpType.mult)
            red = cpool.tile([P, 2 * NSEL], f32, name=f"red{j}")
            nc.vector.reduce_sum(red[:], tmpS[:], axis=mybir.AxisListType.X)
            redi = cpool.tile([P, NSEL], i32, name=f"redi{j}")
            nc.vector.tensor_copy(redi[:], red[:, :NSEL])
            reds.append(red)
            redis.append(redi)
    ots = [cpool.tile([P, D], f32, name=f"ot{j}") for j in range(OWN_TILES)]

    # ---------------- FFN (bf16) + chunked y AllGather + combine ----------
    def emit_combine(g):
        # gather+accumulate chunk g's contributions for the own shard
        with tc.tile_pool(name=f"comb_g{g}", bufs=2,
                          named_scope="combine") as cg:
            for j in range(OWN_TILES):
                for q in range(2):
                    k = q * NG + g
                    yt = cg.tile([P, D], bf16, tag=f"yt{j}_{q}",
                                 name=f"yt{j}_{q}")
                    nc.gpsimd.indirect_dma_start(
                        out=yt[:], out_offset=None, in_=y_all[g][:],
                        in_offset=IndirectOffsetOnAxis(
                            ap=redis[j][:, k:k + 1], axis=0))
                    w = reds[j][:, NSEL + k:NSEL + k + 1]
                    if g == 0 and q == 0:
                        nc.vector.tensor_scalar(
                            ots[j][:], yt[:], w, None,
                            op0=mybir.AluOpType.mult)
                    else:
                        nc.vector.scalar_tensor_tensor(
                            out=ots[j][:], in0=yt[:], scalar=w,
                            in1=ots[j][:], op0=mybir.AluOpType.mult,
                            op1=mybir.AluOpType.add)

    with tc.tile_pool(name="ffn_sb", bufs=1, named_scope="ffn") as fpool, \
         tc.tile_pool(name="ffn_ps", bufs=1, space="PSUM") as fps:
        for g in range(NG):
            t0 = g * TG
            py = [[fps.tile([P, 512], f32, tag=f"py_{t}_{n}",
                            name=f"py_{t}_{n}")
                   for n in range(2)] for t in range(TG)]
            for f in range(NF):
                ph = fps.tile([P, CHK], f32, tag="ph", bufs=2, name="ph")
                for k in range(KD):
                    nc.tensor.matmul(
                        ph[:], lhsT=w1_sb[k][:, f * P:(f + 1) * P],
                        rhs=xgT[k][:, t0 * P:t0 * P + CHK],
                        start=(k == 0), stop=(k == KD - 1))
                hbuf = fpool.tile([P, CHK], bf16, tag="hbuf", bufs=3,
                                  name="hbuf")
                nc.scalar.activation(hbuf[:], ph[:],
                                     mybir.ActivationFunctionType.Relu,
                                     bias=b1_sb[:, f:f + 1], scale=1.0)
                w2f = fpool.tile([P, D], bf16, tag="w2f", bufs=3, name="w2f")
                nc.sync.dma_start(w2f[:], w2d[f * P:(f + 1) * P, :])
                for t in range(TG):
                    for n in range(2):
                        nc.tensor.matmul(
                            py[t][n][:],
                            lhsT=hbuf[:, t * P:(t + 1) * P],
                            rhs=w2f[:, n * 512:(n + 1) * 512],
                            start=(f == 0), stop=(f == NF - 1))
            # add b2 (replicated across partitions) during PSUM drain
            for t in range(TG):
                ysb = fpool.tile([P, D], bf16, tag="ysb", bufs=2, name="ysb")
                for n in range(2):
                    nc.vector.tensor_tensor(
                        out=ysb[:, n * 512:(n + 1) * 512], in0=py[t][n][:],
                        in1=b2_sb[:, n * 512:(n + 1) * 512],
                        op=mybir.AluOpType.add)
                nc.sync.dma_start(y_dram[g][t * P:(t + 1) * P, :], ysb[:])
            # combine the PREVIOUS chunk (its AllGather has landed by now)
            # before this chunk's AG trigger, so the Pool queue isn't blocked
            if g >= 1:
                emit_combine(g - 1)
            # ship this chunk while the next group computes
            nc.gpsimd.collective_compute(
                "AllGather", mybir.AluOpType.bypass,
                replica_groups=[list(range(N_CORES))],
                ins=[y_dram[g][:].opt()],
                outs=[y_all[g][:].opt()],
            )
        emit_combine(NG - 1)
        if dbg is not None:
            nc.sync.dma_start(dbg["dbg_xgT"][:], xgT[0][:])
            for g in range(NG):
                nc.sync.dma_start(dbg["dbg_y"][g * CHK:(g + 1) * CHK, :],
                                  y_dram[g][:])

    for j in range(OWN_TILES):
        nc.sync.dma_start(out[j * P:(j + 1) * P, :], ots[j][:])

    cpool.release()
    persist.release()
    dram.release()


def _host_prep(x, Wg, bg, W1, b1, W2, b2, C):
    xf = np.ascontiguousarray(x.reshape(T, D).astype(np.float32))
    xT = np.ascontiguousarray(xf.T)
    xbf = xf.astype(ml_dtypes.bfloat16)
    bgb = np.tile(bg.astype(np.float32), NT)[None, :].repeat(P, 0)
    bgb = np.ascontiguousarray(bgb)
    l128 = np.triu(np.ones((P, P), np.float32), 1)  # [t', t] = 1 if t' < t
    in_maps = []
    for c in range(N_CORES):
        sel = np.zeros(E, np.float32)
        sel[c] = 1.0
        sel256 = np.ascontiguousarray(np.tile(sel, NT)[None, :].repeat(P, 0))
        ownm = np.zeros((P, OWN_TILES, NT), np.float32)
        for j in range(OWN_TILES):
            ownm[:, j, OWN_TILES * c + j] = 1.0
        in_maps.append({
            "xTs": np.ascontiguousarray(
                xT[:, c * TOK_PER_CORE:(c + 1) * TOK_PER_CORE]),
            "xbf": xbf,
            "w1d": np.ascontiguousarray(W1[c].astype(ml_dtypes.bfloat16)),
            "w2d": np.ascontiguousarray(W2[c].astype(ml_dtypes.bfloat16)),
            "wgd": np.ascontiguousarray(Wg.astype(np.float32)),
            "bgb": bgb,
            "b1pm": np.ascontiguousarray(
                b1[c].astype(np.float32).reshape(NF, P).T),
            "b2pm": np.ascontiguousarray(
                np.tile(b2[c].astype(np.float32)[None, :], (P, 1))),
            "sel256": sel256,
            "l128d": l128,
            "ownmd": np.ascontiguousarray(ownm.reshape(P, OWN_TILES * NT)),
        })
    return in_maps


def _capacity(x, Wg, bg):
    xf = x.reshape(T, D).astype(np.float32)
    logits = xf @ Wg.astype(np.float32) + bg.astype(np.float32)
    part = np.partition(logits, E - 2, axis=-1)
    m2 = part[:, E - 2:E - 1]
    counts = (logits >= m2).sum(0)
    return int(np.ceil((counts.max() + 16) / CHK) * CHK)


LAST_RESULT = None


def kernel(x, Wg, bg, W1, b1, W2, b2):
    global LAST_RESULT
    from concourse.bass_utils import run_bass_kernel_spmd

    x = np.asarray(x)
    C = _capacity(x, np.asarray(Wg), np.asarray(bg))
    if C not in _cache:
        _cache[C] = build_module(C)
    nc = _cache[C]
    in_maps = _host_prep(x, np.asarray(Wg), np.asarray(bg), np.asarray(W1),
                         np.asarray(b1), np.asarray(W2), np.asarray(b2), C)
    trace = bool(os.environ.get("BASS_TRACE"))
    if trace:
        _setup_axon_profile_hook()
    res = run_bass_kernel_spmd(nc, in_maps, core_ids=list(range(N_CORES)),
                               trace=trace)
    LAST_RESULT = res
    out = np.empty((T, D), np.float32)
    for c in range(N_CORES):
        out[c * TOK_PER_CORE:(c + 1) * TOK_PER_CORE] = res.results[c]["out"]
    return out.reshape(B, S, D)


def _setup_axon_profile_hook():
    """Provide antenv.axon_hooks (missing in this image) so trace=True works."""
    import types
    try:
        import antenv
        if "antenv.axon_hooks" not in sys.modules:
            hooks = types.ModuleType("antenv.axon_hooks")
            hooks._hook = None
            hooks.set_axon_ntff_profile_hook = \
                lambda h: setattr(hooks, "_hook", h)
            hooks.get_axon_ntff_profile_hook = lambda: hooks._hook
            sys.modules["antenv.axon_hooks"] = hooks
            antenv.axon_hooks = hooks
            from trn_agent_boot.trn_boot import _ntff_profile_via_ctypes
            hooks.set_axon_ntff_profile_hook(
                _ntff_profile_via_ctypes("/opt/axon/libaxon_pjrt.so"))
    except Exception as e:  # profiling is best-effort
        print(f"profile hook setup failed: {e}", file=sys.stderr)


# revision 41
# speedup vs baseline: 1.0745x; 1.0321x over previous
"""Expert-parallel top-2 MoE kernel for 8 Trainium2 NeuronCores.

Strategy (expert-parallel, sparse dispatch, per the sharding hint):
  - Router sharded over cores: core c computes fp32 logits for its 512-token
    shard on the TensorEngine (Wg stationary), AllGathers them (contiguous
    [E, tok] layout, transposed back on the PE) so every core holds identical
    logits for all 4096 tokens; softmax/top-2 on-device.
  - Core c owns expert c: slot positions come from matmul-based exclusive
    cumsums; the slot->token map is built with a single indirect-scatter DMA
    (token ids scattered to their slot positions; unrouted tokens get an
    out-of-bounds position and are dropped via bounds_check). Routed tokens
    are gathered via indirect DMA, transposed on the TensorEngine, and run
    through the two-layer FFN in bf16 (capacity padded to a multiple of 384).
  - Unscaled expert outputs (bf16) are AllGathered chunk-by-chunk into one
    shared [NG*8*CHK, D] tensor (overlapped with the FFN); each core combines
    the top-2 contributions for its own 512-token shard with two indirect
    row-gathers per 128-token tile + gate-weighted sum in fp32.

Numerics: router fp32 (top-2 selection fidelity), FFN bf16 with fp32
accumulation in PSUM, combine in fp32.
"""

import os
import sys

import numpy as np

for _p in ("/opt/trn_rl_repo",):
    if _p not in sys.path:
        sys.path.append(_p)

import ml_dtypes

import concourse.bass as bass
import concourse.mybir as mybir
import concourse.tile as tile
from concourse import bacc
from concourse.bass import IndirectOffsetOnAxis
from concourse.masks import make_identity

# Problem shapes (fixed per spec)
B, S, D, E = 2, 2048, 1024, 8
T = B * S          # 4096 tokens
F = 4 * D          # 4096 ffn dim
P = 128            # partitions
NT = T // P        # 32 token tiles
KD = D // P        # 8 contraction tiles over D
NF = F // P        # 32 f tiles
TOK_PER_CORE = T // E   # 512
OWN_TILES = TOK_PER_CORE // P  # 4
N_CORES = E
TG = 3                       # slot tiles per FFN group
CHK = TG * P                 # 384: slot chunk for the chunked AllGather
BIGPOS = float(1 << 20)      # scatter position for unrouted tokens (dropped)

f32 = mybir.dt.float32
bf16 = mybir.dt.bfloat16
f16 = mybir.dt.float16
i32 = mybir.dt.int32
u32 = mybir.dt.uint32

_cache = {}


def build_module(C: int, debug_out: bool = False):
    """Build the SPMD Bass module for capacity C (multiple of 384)."""
    assert C % CHK == 0
    ST = C // P  # slot tiles per expert

    nc = bacc.Bacc("TRN2", target_bir_lowering=False, debug=False,
                   num_devices=N_CORES)

    # ---- I/O ----
    xTs = nc.dram_tensor("xTs", [D, TOK_PER_CORE], f32,
                         kind="ExternalInput").ap()
    xbf = nc.dram_tensor("xbf", [T, D], bf16, kind="ExternalInput").ap()
    w1d = nc.dram_tensor("w1d", [D, F], bf16, kind="ExternalInput").ap()
    w2d = nc.dram_tensor("w2d", [F, D], bf16, kind="ExternalInput").ap()
    wgd = nc.dram_tensor("wgd", [D, E], f32, kind="ExternalInput").ap()
    bgb = nc.dram_tensor("bgb", [P, NT * E], f32, kind="ExternalInput").ap()
    b1pm = nc.dram_tensor("b1pm", [P, NF], f32, kind="ExternalInput").ap()
    b2pm = nc.dram_tensor("b2pm", [P, D], f32, kind="ExternalInput").ap()
    sel256 = nc.dram_tensor("sel256", [P, NT * E], f32,
                            kind="ExternalInput").ap()
    l128d = nc.dram_tensor("l128d", [P, P], f32, kind="ExternalInput").ap()
    ownmd = nc.dram_tensor("ownmd", [P, OWN_TILES * NT], f32,
                           kind="ExternalInput").ap()
    out = nc.dram_tensor("out", [TOK_PER_CORE, D], f32,
                         kind="ExternalOutput").ap()
    dbg = None
    if debug_out:
        dbg = {
            "dbg_l": nc.dram_tensor("dbg_l", [P, NT * E], f32,
                                    kind="ExternalOutput").ap(),
            "dbg_pos": nc.dram_tensor("dbg_pos", [P, NT * E], f32,
                                      kind="ExternalOutput").ap(),
            "dbg_posm": nc.dram_tensor("dbg_posm", [P, NT], f32,
                                       kind="ExternalOutput").ap(),
            "dbg_idx": nc.dram_tensor("dbg_idx", [P, C // P], i32,
                                      kind="ExternalOutput").ap(),
            "dbg_xgT": nc.dram_tensor("dbg_xgT", [P, C], bf16,
                                      kind="ExternalOutput").ap(),
            "dbg_y": nc.dram_tensor("dbg_y", [C, D], bf16,
                                    kind="ExternalOutput").ap(),
            "dbg_S": nc.dram_tensor("dbg_S", [P, 4 * (C // CHK) * NT], f32,
                                    kind="ExternalOutput").ap(),
        }

    with tile.TileContext(nc) as tc:
        _emit(tc, C, ST, xTs, xbf, w1d, w2d, wgd, bgb, b1pm, b2pm, sel256,
              l128d, ownmd, out, dbg)

    nc.compile()
    return nc


def _emit(tc, C, ST, xTs, xbf, w1d, w2d, wgd, bgb, b1pm, b2pm, sel256,
          l128d, ownmd, out, dbg=None):
    nc = tc.nc
    NE = NT * E  # 256
    NG = ST // TG  # number of FFN groups / y-AllGather chunks
    GBLK = N_CORES * CHK  # rows per AllGather chunk in y_big

    # ---------------- persistent pools ----------------
    persist = tc.alloc_tile_pool(name="persist", bufs=1)
    dram = tc.alloc_tile_pool(name="dram", bufs=1, space="DRAM")

    # constants / weights resident in SBUF
    wg_sb = persist.tile([P, KD, E], f32, name="wg_sb")
    nc.sync.dma_start(wg_sb[:], wgd.rearrange("(k p) e -> p k e", p=P))
    bg_sb = persist.tile([P, NE], f32, name="bg_sb")
    nc.sync.dma_start(bg_sb[:], bgb[:])
    sel_sb = persist.tile([P, NE], f32, name="sel_sb")
    nc.sync.dma_start(sel_sb[:], sel256[:])
    l128_sb = persist.tile([P, P], f32, name="l128_sb")
    nc.sync.dma_start(l128_sb[:], l128d[:])
    ownm_sb = persist.tile([P, OWN_TILES * NT], f32, name="ownm_sb")
    nc.sync.dma_start(ownm_sb[:], ownmd[:])
    b1_sb = persist.tile([P, NF], f32, name="b1_sb")
    nc.sync.dma_start(b1_sb[:], b1pm[:])
    b2_sb = persist.tile([P, D], f32, name="b2_sb")
    nc.sync.dma_start(b2_sb[:], b2pm[:])
    ident = persist.tile([P, P], bf16, name="ident")
    make_identity(nc, ident[:])
    identf = persist.tile([P, P], f32, name="identf")
    make_identity(nc, identf[:])
    ones_col = persist.tile([P, 1], f32, name="ones_col")
    nc.vector.memset(ones_col[:], 1.0)
    ones_row = persist.tile([1, P], f32, name="ones_row")
    nc.vector.memset(ones_row[:], 1.0)

    # w1/w2 tiles allocated here; the 16MB of loads are issued after the
    # router's small DMAs so they don't head-of-line block the critical path.
    # W2 is SBUF-resident so the FFN phase has no weight streaming competing
    # with the y-AllGather HBM traffic.
    w1_sb = [persist.tile([P, F], bf16, name=f"w1_sb{k}") for k in range(KD)]
    w2_sb = [persist.tile([P, D], bf16, name=f"w2_sb{f}") for f in range(NF)]

    # router / dispatch state kept for the combine phase
    exp_all = persist.tile([P, NE], f32, name="exp_all")    # exp(logits)
    m8_all = persist.tile([P, NE], f32, name="m8_all")      # per-tile top8 of exp
    r_all = persist.tile([P, NT], f32, name="r_all")        # 1/sum(exp)
    pos_all = persist.tile([P, NE], f32, name="pos_all")    # excl cumsum per expert
    ind_all = persist.tile([P, NE], f32, name="ind_all")    # top2 indicator
    ei_all = persist.tile([P, NE], u32, name="ei_all")      # top8 expert indices
    idx_sb = persist.tile([P, ST], i32, name="idx_sb")      # slot -> token id

    xgT = [persist.tile([P, C], bf16, name=f"xgT{d}") for d in range(KD)]

    l_dram = dram.tile([E, TOK_PER_CORE], f32, name="l_dram")
    lg_dram = dram.tile([N_CORES * E, TOK_PER_CORE], f32, addr_space="Shared",
                        name="lg_dram")
    y_dram = [dram.tile([CHK, D], bf16, name=f"y_dram{g}") for g in range(NG)]
    # one Shared AllGather output per chunk (Shared tensors allow one writer)
    y_all = [dram.tile([N_CORES * CHK, D], bf16, addr_space="Shared",
                       name=f"y_all{g}") for g in range(NG)]

    # ---------------- router (sharded + AllGather) ----------------
    with tc.tile_pool(name="router_sb", bufs=1, named_scope="router") as rpool, \
         tc.tile_pool(name="router_ps", bufs=1, space="PSUM") as rps:
        xs = rpool.tile([P, KD, TOK_PER_CORE], f32, name="xs")
        nc.sync.dma_start(xs[:], xTs.rearrange("(k p) t -> p k t", p=P))
        lT = rps.tile([E, TOK_PER_CORE], f32, name="lT")
        for k in range(KD):
            nc.tensor.matmul(lT[:], lhsT=wg_sb[:, k, :], rhs=xs[:, k, :],
                             start=(k == 0), stop=(k == KD - 1))
        lt_sb = rpool.tile([E, TOK_PER_CORE], f32, name="lt_sb")
        nc.vector.tensor_copy(lt_sb[:], lT[:])
        nc.sync.dma_start(l_dram[:], lt_sb[:])
        nc.gpsimd.collective_compute(
            "AllGather", mybir.AluOpType.bypass,
            replica_groups=[list(range(N_CORES))],
            ins=[l_dram[:].opt()], outs=[lg_dram[:].opt()],
        )
        # lg[(c e), tok_local]; token t*P+p has t = c*4 + t4,
        # tok_local = t4*P + p. Transpose each 128-token block on the PE.
        lg_sb = rpool.tile([N_CORES * E, TOK_PER_CORE], f32, name="lg_sb")
        nc.sync.dma_start(lg_sb[:], lg_dram[:])
        for k in range(KD):
            nc.sync.dma_start(w1_sb[k][:], w1d[k * P:(k + 1) * P, :])
        for f in range(NF):
            nc.sync.dma_start(w2_sb[f][:], w2d[f * P:(f + 1) * P, :])
        l_all = rpool.tile([P, NE], f32, name="l_all")
        l_all4 = l_all[:].rearrange("p (c t4 e) -> p c t4 e", c=N_CORES, t4=4)
        for t4 in range(4):
            ptp = rps.tile([P, N_CORES * E], f32, tag="ptp", bufs=2,
                           name="ptp")
            nc.tensor.transpose(ptp[:], lg_sb[:, t4 * P:(t4 + 1) * P],
                                identf[:N_CORES * E, :N_CORES * E])
            nc.vector.tensor_copy(
                l_all4[:, :, t4, :],
                ptp[:].rearrange("p (c e) -> p c e", e=E))
        nc.vector.tensor_add(l_all[:], l_all[:], bg_sb[:])
        if dbg is not None:
            nc.sync.dma_start(dbg["dbg_l"][:], l_all[:])
        nc.scalar.activation(exp_all[:], l_all[:],
                             mybir.ActivationFunctionType.Exp)
        # sums and reciprocal per token
        s_all = rpool.tile([P, NT], f32, name="s_all")
        nc.vector.reduce_sum(s_all[:], exp_all[:].rearrange(
            "p (t e) -> p t e", e=E), axis=mybir.AxisListType.X)
        nc.vector.reciprocal(r_all[:], s_all[:])
        # per-tile top8 (indices for the combine are extracted later, off
        # the dispatch critical path), then one batched top-2 indicator
        for tt in range(NT):
            sl = slice(tt * E, (tt + 1) * E)
            nc.vector.max(out=m8_all[:, sl], in_=exp_all[:, sl])
        m83r = m8_all[:].rearrange("p (t e) -> p t e", e=E)
        nc.vector.tensor_tensor(
            out=ind_all[:].rearrange("p (t e) -> p t e", e=E),
            in0=exp_all[:].rearrange("p (t e) -> p t e", e=E),
            in1=m83r[:, :, 1][:, :, None].to_broadcast([P, NT, E]),
            op=mybir.AluOpType.is_ge)

    # ---------------- dispatch: positions + scatter slot->token map --------
    with tc.tile_pool(name="disp_sb", bufs=1, named_scope="dispatch") as dpool, \
         tc.tile_pool(name="disp_ps", bufs=1, space="PSUM") as dps:
        # per-tile totals: ptot[0, (t e)] = sum_p ind_all[p, (t e)]
        ptot = dps.tile([1, NE], f32, name="ptot")
        nc.tensor.matmul(ptot[:], lhsT=ones_col[:], rhs=ind_all[:],
                         start=True, stop=True)
        tot_flat = dpool.tile([1, NE], f32, name="tot_flat")
        nc.vector.tensor_copy(tot_flat[:], ptot[:])
        # reshape [1, NT*E] -> [NT, E] via sbuf-to-sbuf DMA
        tot32 = dpool.tile([NT, E], f32, name="tot32")
        nc.sync.dma_start(tot32[:], tot_flat[:])
        # exclusive cumsum over tiles: strict-lower matmul
        pofs = dps.tile([NT, E], f32, name="pofs")
        nc.tensor.matmul(pofs[:], lhsT=l128_sb[:NT, :NT], rhs=tot32[:],
                         start=True, stop=True)
        ofs32 = dpool.tile([NT, E], f32, name="ofs32")
        nc.vector.tensor_copy(ofs32[:], pofs[:])
        ofs_flat = dpool.tile([1, NE], f32, name="ofs_flat")
        nc.sync.dma_start(ofs_flat[:], ofs32[:])
        # positions: local excl cumsum (over partitions) + tile offset
        ppos = dps.tile([P, NE], f32, name="ppos")
        nc.tensor.matmul(ppos[:], lhsT=l128_sb[:], rhs=ind_all[:],
                         start=True, stop=False)
        nc.tensor.matmul(ppos[:], lhsT=ones_row[:], rhs=ofs_flat[:],
                         start=False, stop=True)
        nc.vector.tensor_copy(pos_all[:], ppos[:])

        # my expert's positions / indicator
        tmp = dpool.tile([P, NE], f32, name="tmp")
        nc.vector.tensor_mul(tmp[:], pos_all[:], sel_sb[:])
        pos_e = dpool.tile([P, NT], f32, name="pos_e")
        nc.vector.reduce_sum(pos_e[:], tmp[:].rearrange(
            "p (t e) -> p t e", e=E), axis=mybir.AxisListType.X)
        nc.vector.tensor_mul(tmp[:], ind_all[:], sel_sb[:])
        ind_e = dpool.tile([P, NT], f32, name="ind_e")
        nc.vector.reduce_sum(ind_e[:], tmp[:].rearrange(
            "p (t e) -> p t e", e=E), axis=mybir.AxisListType.X)
        # masked positions: ind ? pos : BIGPOS (matches no slot)
        pos_m = dpool.tile([P, NT], f32, name="pos_m")
        nc.vector.tensor_scalar_add(pos_m[:], pos_e[:], -BIGPOS)
        nc.vector.tensor_mul(pos_m[:], pos_m[:], ind_e[:])
        nc.vector.tensor_scalar_add(pos_m[:], pos_m[:], BIGPOS)
        # remapped position pos2 = (pos & 127)*ST + (pos >> 7), so the final
        # departition DMA is contiguous per partition. BIGPOS remaps to
        # 8192 >= C (fp16-exact) and matches no slot.
        pos_i = dpool.tile([P, NT], i32, name="pos_i")
        nc.vector.tensor_copy(pos_i[:], pos_m[:])
        ph1 = dpool.tile([P, NT], i32, name="ph1")
        nc.vector.tensor_scalar(ph1[:], pos_i[:], 7, None,
                                op0=mybir.AluOpType.logical_shift_right)
        ph2 = dpool.tile([P, NT], i32, name="ph2")
        nc.vector.tensor_scalar(ph2[:], pos_i[:], 127, None,
                                op0=mybir.AluOpType.bitwise_and)
        nc.vector.tensor_scalar(ph2[:], ph2[:], ST, None,
                                op0=mybir.AluOpType.mult)
        pos2 = dpool.tile([P, NT], i32, name="pos2")
        nc.vector.tensor_add(pos2[:], ph1[:], ph2[:])
        pos_mh = dpool.tile([P, NT], f32, name="pos_mh")
        nc.vector.tensor_copy(pos_mh[:], pos2[:])

        # slot->token map via fp16 one-hot + rank-2 matmul:
        #   Pt[p, c] = (c == pos2[p, tt]);  token id = p + 128*tt, so
        #   accumulate [p-part; 128*tt-part] over tiles with a 2-col lhsT.
        # (These iota/const tiles have no deps and get scheduled early.)
        iotaC_i = dpool.tile([P, C], i32, name="iotaC_i")
        nc.gpsimd.iota(iotaC_i[:], pattern=[[1, C]], base=0,
                       channel_multiplier=0)
        iotaC_h = dpool.tile([P, C], f16, name="iotaC_h")
        nc.vector.tensor_copy(iotaC_h[:], iotaC_i[:])
        tokp_i = dpool.tile([P, 1], i32, name="tokp_i")
        nc.gpsimd.iota(tokp_i[:], pattern=[[0, 1]], base=0,
                       channel_multiplier=1)
        tokt_i = dpool.tile([P, NT], i32, name="tokt_i")
        nc.gpsimd.iota(tokt_i[:], pattern=[[P, NT]], base=0,
                       channel_multiplier=0)
        tok2 = dpool.tile([P, NT, 2], f16, name="tok2")
        nc.vector.tensor_copy(tok2[:, :, 0],
                              tokp_i[:, 0:1].to_broadcast([P, NT]))
        nc.vector.tensor_copy(tok2[:, :, 1], tokt_i[:])

        NCH = (C + 511) // 512
        pid_ps = [dps.tile([2, min(512, C - ch * 512)], f32,
                           name=f"pid{ch}") for ch in range(NCH)]
        for tt in range(NT):
            Pt = dpool.tile([P, C], f16, tag="Pt", bufs=4, name="Pt")
            nc.vector.tensor_scalar(Pt[:], iotaC_h[:],
                                    pos_mh[:, tt:tt + 1], None,
                                    op0=mybir.AluOpType.is_equal)
            for ch in range(NCH):
                c0 = ch * 512
                c1 = min(c0 + 512, C)
                nc.tensor.matmul(pid_ps[ch][:], lhsT=tok2[:, tt, :],
                                 rhs=Pt[:, c0:c1],
                                 start=(tt == 0), stop=(tt == NT - 1))
        pid_sb = dpool.tile([2, C], f32, name="pid_sb")
        for ch in range(NCH):
            c0 = ch * 512
            c1 = min(c0 + 512, C)
            nc.vector.tensor_copy(pid_sb[:, c0:c1], pid_ps[ch][:])
        # departition [2, C] -> [P, 2, ST] (contiguous per partition), then
        # token id = p-part + 128*tt-part
        idx2f = dpool.tile([P, 2, ST], f32, name="idx2f")
        for r in range(2):
            nc.sync.dma_start(idx2f[:, r, :], pid_sb[r:r + 1, :])
        idxf = dpool.tile([P, ST], f32, name="idxf")
        nc.vector.tensor_add(idxf[:], idx2f[:, 0, :], idx2f[:, 1, :])
        nc.vector.tensor_copy(idx_sb[:], idxf[:])
        if dbg is not None:
            nc.sync.dma_start(dbg["dbg_pos"][:], pos_all[:])
            nc.sync.dma_start(dbg["dbg_posm"][:], pos_m[:])
            nc.sync.dma_start(dbg["dbg_idx"][:], idx_sb[:])

        # gather tokens (bf16) and transpose into xgT
        with tc.tile_pool(name="gat_sb", bufs=2) as gpool, \
             tc.tile_pool(name="gat_ps", bufs=2, space="PSUM") as gps:
            for s in range(ST):
                xg = gpool.tile([P, D], bf16, tag="xg", name="xg")
                nc.gpsimd.indirect_dma_start(
                    out=xg[:], out_offset=None, in_=xbf[:],
                    in_offset=IndirectOffsetOnAxis(ap=idx_sb[:, s:s + 1],
                                                   axis=0),
                )
                for d in range(KD):
                    pt = gps.tile([P, P], bf16, tag="pt", name="pt")
                    nc.tensor.transpose(pt[:], xg[:, d * P:(d + 1) * P],
                                        ident[:])
                    # alternate the PSUM drains between DVE and ScalarE so
                    # neither engine gates the gather/transpose pipeline
                    if d % 2 == 0:
                        nc.vector.tensor_copy(
                            xgT[d][:, s * P:(s + 1) * P], pt[:])
                    else:
                        nc.scalar.activation(
                            xgT[d][:, s * P:(s + 1) * P], pt[:],
                            mybir.ActivationFunctionType.Copy)

    # ---------------- combine planes (needs only router/dispatch state) ----
    cpool = tc.alloc_tile_pool(name="comb_sb", bufs=1)
    with tc.tile_pool(name="comb_tmp", bufs=2, named_scope="combine") as ctmp:
        # top-8 indices (only top-2 used), off the dispatch critical path
        for tt in range(NT):
            sl = slice(tt * E, (tt + 1) * E)
            nc.vector.max_index(out=ei_all[:, sl], in_max=m8_all[:, sl],
                                in_values=exp_all[:, sl])
        # expert ids of top-1/top-2 as f32
        e1f = ctmp.tile([P, NT], f32, tag="e1f", bufs=1, name="e1f")
        e2f = ctmp.tile([P, NT], f32, tag="e2f", bufs=1, name="e2f")
        ei3 = ei_all[:].rearrange("p (t e) -> p t e", e=E)
        nc.vector.tensor_copy(e1f[:], ei3[:, :, 0])
        nc.vector.tensor_copy(e2f[:], ei3[:, :, 1])
        ioz = ctmp.tile([P, NE], i32, tag="ioz", bufs=1, name="ioz")
        nc.gpsimd.iota(ioz[:].rearrange("p (t e) -> p t e", e=E),
                       pattern=[[0, NT], [1, E]], base=0, channel_multiplier=0)
        iof = ctmp.tile([P, NE], f32, tag="iof", bufs=1, name="iof")
        nc.vector.tensor_copy(iof[:], ioz[:])
        m83 = m8_all[:].rearrange("p (t e) -> p t e", e=E)

        # Selection stack S: for each (q, g) a row-index plane into y_all[g]
        # (rows not in chunk g point at row 0) and a masked gate-weight
        # plane; one masked reduce per owner tile pulls all 12 values.
        NSEL = 2 * NG
        S = ctmp.tile([P, 2 * NSEL, NT], f32, bufs=1, name="S")
        for q, ef in ((0, e1f), (1, e2f)):
            oh = ctmp.tile([P, NE], f32, tag=f"oh{q}", bufs=1, name=f"oh{q}")
            nc.vector.tensor_tensor(
                out=oh[:].rearrange("p (t e) -> p t e", e=E),
                in0=iof[:].rearrange("p (t e) -> p t e", e=E),
                in1=ef[:, :, None].to_broadcast([P, NT, E]),
                op=mybir.AluOpType.is_equal)
            nc.vector.tensor_mul(oh[:], oh[:], pos_all[:])
            slot = ctmp.tile([P, NT], f32, tag=f"slot{q}", bufs=1,
                             name=f"slot{q}")
            nc.vector.reduce_sum(slot[:], oh[:].rearrange(
                "p (t e) -> p t e", e=E), axis=mybir.AxisListType.X)
            # chunk id g = (slot>=CHK) + (slot>=2*CHK) + ...
            gch = ctmp.tile([P, NT], f32, tag=f"gch{q}", bufs=1,
                            name=f"gch{q}")
            nc.vector.tensor_scalar(gch[:], slot[:], float(CHK), None,
                                    op0=mybir.AluOpType.is_ge)
            for gg in range(2, NG):
                t2 = ctmp.tile([P, NT], f32, tag="t2", name="t2")
                nc.vector.tensor_scalar(t2[:], slot[:], float(CHK * gg), None,
                                        op0=mybir.AluOpType.is_ge)
                nc.vector.tensor_add(gch[:], gch[:], t2[:])
            # in-chunk row: e*CHK + (slot - g*CHK)
            base = ctmp.tile([P, NT], f32, tag=f"base{q}", bufs=1,
                             name=f"base{q}")
            nc.vector.scalar_tensor_tensor(
                out=base[:], in0=ef[:], scalar=float(CHK), in1=slot[:],
                op0=mybir.AluOpType.mult, op1=mybir.AluOpType.add)
            gv = ctmp.tile([P, NT], f32, tag=f"gv{q}", bufs=1, name=f"gv{q}")
            nc.vector.tensor_tensor(out=gv[:], in0=m83[:, :, q], in1=r_all[:],
                                    op=mybir.AluOpType.mult)
            for gg in range(NG):
                k = q * NG + gg
                eq = ctmp.tile([P, NT], f32, tag="eq", name="eq")
                nc.vector.tensor_scalar(eq[:], gch[:], float(gg), None,
                                        op0=mybir.AluOpType.is_equal)
                nc.vector.tensor_scalar_add(S[:, k, :], base[:],
                                            float(-CHK * gg))
                nc.vector.tensor_mul(S[:, k, :], S[:, k, :], eq[:])
                nc.vector.tensor_mul(S[:, NSEL + k, :], eq[:], gv[:])

        if dbg is not None:
            nc.sync.dma_start(
                dbg["dbg_S"][:].rearrange("p (k t) -> p k t", k=2 * NSEL),
                S[:])
        # per-owner-tile row indices and weights, ready before the FFN
        reds = []
        redis = []
        for j in range(OWN_TILES):
            own = ownm_sb[:, j * NT:(j + 1) * NT]
            tmpS = ctmp.tile([P, 2 * NSEL, NT], f32, tag="tmpS", bufs=2,
                             name="tmpS")
            nc.vector.tensor_tensor(
                out=tmpS[:], in0=S[:],
                in1=own[:, None, :].to_broadcast([P, 2 * NSEL, NT]),
                op=mybir.AluOpType.mult)
            red = cpool.tile([P, 2 * NSEL], f32, name=f"red{j}")
            nc.vector.reduce_sum(red[:], tmpS[:], axis=mybir.AxisListType.X)
            redi = cpool.tile([P, NSEL], i32, name=f"redi{j}")
            nc.vector.tensor_copy(redi[:], red[:, :NSEL])
            reds.append(red)
            redis.append(redi)
    ots = [cpool.tile([P, D], f32, name=f"ot{j}") for j in range(OWN_TILES)]

    # ---------------- FFN (bf16) + chunked y AllGather + combine ----------
    def emit_combine(g):
        # gather+accumulate chunk g's contributions for the own shard
        with tc.tile_pool(name=f"comb_g{g}", bufs=2,
                          named_scope="combine") as cg:
            for j in range(OWN_TILES):
                for q in range(2):
                    k = q * NG + g
                    yt = cg.tile([P, D], bf16, tag="yt", bufs=4, name="yt")
                    nc.gpsimd.indirect_dma_start(
                        out=yt[:], out_offset=None, in_=y_all[g][:],
                        in_offset=IndirectOffsetOnAxis(
                            ap=redis[j][:, k:k + 1], axis=0))
                    w = reds[j][:, NSEL + k:NSEL + k + 1]
                    if g == 0 and q == 0:
                        nc.vector.tensor_scalar(
                            ots[j][:], yt[:], w, None,
                            op0=mybir.AluOpType.mult)
                    else:
                        nc.vector.scalar_tensor_tensor(
                            out=ots[j][:], in0=yt[:], scalar=w,
                            in1=ots[j][:], op0=mybir.AluOpType.mult,
                            op1=mybir.AluOpType.add)

    with tc.tile_pool(name="ffn_sb", bufs=1, named_scope="ffn") as fpool, \
         tc.tile_pool(name="ffn_ps", bufs=1, space="PSUM") as fps:
        for g in range(NG):
            t0 = g * TG
            py = [[fps.tile([P, 512], f32, tag=f"py_{t}_{n}",
                            name=f"py_{t}_{n}")
                   for n in range(2)] for t in range(TG)]
            for f in range(NF):
                ph = fps.tile([P, CHK], f32, tag="ph", bufs=2, name="ph")
                for k in range(KD):
                    nc.tensor.matmul(
                        ph[:], lhsT=w1_sb[k][:, f * P:(f + 1) * P],
                        rhs=xgT[k][:, t0 * P:t0 * P + CHK],
                        start=(k == 0), stop=(k == KD - 1))
                hbuf = fpool.tile([P, CHK], bf16, tag="hbuf", bufs=3,
                                  name="hbuf")
                nc.scalar.activation(hbuf[:], ph[:],
                                     mybir.ActivationFunctionType.Relu,
                                     bias=b1_sb[:, f:f + 1], scale=1.0)
                for t in range(TG):
                    for n in range(2):
                        nc.tensor.matmul(
                            py[t][n][:],
                            lhsT=hbuf[:, t * P:(t + 1) * P],
                            rhs=w2_sb[f][:, n * 512:(n + 1) * 512],
                            start=(f == 0), stop=(f == NF - 1))
            # add b2 (replicated across partitions) during PSUM drain
            for t in range(TG):
                ysb = fpool.tile([P, D], bf16, tag="ysb", bufs=2, name="ysb")
                for n in range(2):
                    nc.vector.tensor_tensor(
                        out=ysb[:, n * 512:(n + 1) * 512], in0=py[t][n][:],
                        in1=b2_sb[:, n * 512:(n + 1) * 512],
                        op=mybir.AluOpType.add)
                nc.sync.dma_start(y_dram[g][t * P:(t + 1) * P, :], ysb[:])
            # combine the PREVIOUS chunk (its AllGather has landed by now)
            # before this chunk's AG trigger, so the Pool queue isn't blocked
            if g >= 1:
                emit_combine(g - 1)
            # ship this chunk while the next group computes
            nc.gpsimd.collective_compute(
                "AllGather", mybir.AluOpType.bypass,
                replica_groups=[list(range(N_CORES))],
                ins=[y_dram[g][:].opt()],
                outs=[y_all[g][:].opt()],
            )
        emit_combine(NG - 1)
        if dbg is not None:
            nc.sync.dma_start(dbg["dbg_xgT"][:], xgT[0][:])
            for g in range(NG):
                nc.sync.dma_start(dbg["dbg_y"][g * CHK:(g + 1) * CHK, :],
                                  y_dram[g][:])

    for j in range(OWN_TILES):
        nc.sync.dma_start(out[j * P:(j + 1) * P, :], ots[j][:])

    cpool.release()
    persist.release()
    dram.release()


def _host_prep(x, Wg, bg, W1, b1, W2, b2, C):
    xf = np.ascontiguousarray(x.reshape(T, D).astype(np.float32))
    xT = np.ascontiguousarray(xf.T)
    xbf = xf.astype(ml_dtypes.bfloat16)
    bgb = np.tile(bg.astype(np.float32), NT)[None, :].repeat(P, 0)
    bgb = np.ascontiguousarray(bgb)
    l128 = np.triu(np.ones((P, P), np.float32), 1)  # [t', t] = 1 if t' < t
    in_maps = []
    for c in range(N_CORES):
        sel = np.zeros(E, np.float32)
        sel[c] = 1.0
        sel256 = np.ascontiguousarray(np.tile(sel, NT)[None, :].repeat(P, 0))
        ownm = np.zeros((P, OWN_TILES, NT), np.float32)
        for j in range(OWN_TILES):
            ownm[:, j, OWN_TILES * c + j] = 1.0
        in_maps.append({
            "xTs": np.ascontiguousarray(
                xT[:, c * TOK_PER_CORE:(c + 1) * TOK_PER_CORE]),
            "xbf": xbf,
            "w1d": np.ascontiguousarray(W1[c].astype(ml_dtypes.bfloat16)),
            "w2d": np.ascontiguousarray(W2[c].astype(ml_dtypes.bfloat16)),
            "wgd": np.ascontiguousarray(Wg.astype(np.float32)),
            "bgb": bgb,
            "b1pm": np.ascontiguousarray(
                b1[c].astype(np.float32).reshape(NF, P).T),
            "b2pm": np.ascontiguousarray(
                np.tile(b2[c].astype(np.float32)[None, :], (P, 1))),
            "sel256": sel256,
            "l128d": l128,
            "ownmd": np.ascontiguousarray(ownm.reshape(P, OWN_TILES * NT)),
        })
    return in_maps


def _capacity(x, Wg, bg):
    xf = x.reshape(T, D).astype(np.float32)
    logits = xf @ Wg.astype(np.float32) + bg.astype(np.float32)
    part = np.partition(logits, E - 2, axis=-1)
    m2 = part[:, E - 2:E - 1]
    counts = (logits >= m2).sum(0)
    return int(np.ceil((counts.max() + 16) / CHK) * CHK)


LAST_RESULT = None


def kernel(x, Wg, bg, W1, b1, W2, b2):
    global LAST_RESULT
    from concourse.bass_utils import run_bass_kernel_spmd

    x = np.asarray(x)
    C = _capacity(x, np.asarray(Wg), np.asarray(bg))
    if C not in _cache:
        _cache[C] = build_module(C)
    nc = _cache[C]
    in_maps = _host_prep(x, np.asarray(Wg), np.asarray(bg), np.asarray(W1),
                         np.asarray(b1), np.asarray(W2), np.asarray(b2), C)
    trace = bool(os.environ.get("BASS_TRACE"))
    if trace:
        _setup_axon_profile_hook()
    res = run_bass_kernel_spmd(nc, in_maps, core_ids=list(range(N_CORES)),
                               trace=trace)
    LAST_RESULT = res
    out = np.empty((T, D), np.float32)
    for c in range(N_CORES):
        out[c * TOK_PER_CORE:(c + 1) * TOK_PER_CORE] = res.results[c]["out"]
    return out.reshape(B, S, D)


def _setup_axon_profile_hook():
    """Provide antenv.axon_hooks (missing in this image) so trace=True works."""
    import types
    try:
        import antenv
        if "antenv.axon_hooks" not in sys.modules:
            hooks = types.ModuleType("antenv.axon_hooks")
            hooks._hook = None
            hooks.set_axon_ntff_profile_hook = \
                lambda h: setattr(hooks, "_hook", h)
            hooks.get_axon_ntff_profile_hook = lambda: hooks._hook
            sys.modules["antenv.axon_hooks"] = hooks
            antenv.axon_hooks = hooks
            from trn_agent_boot.trn_boot import _ntff_profile_via_ctypes
            hooks.set_axon_ntff_profile_hook(
                _ntff_profile_via_ctypes("/opt/axon/libaxon_pjrt.so"))
    except Exception as e:  # profiling is best-effort
        print(f"profile hook setup failed: {e}", file=sys.stderr)


# revision 44
# speedup vs baseline: 1.1417x; 1.0626x over previous
"""Expert-parallel top-2 MoE kernel for 8 Trainium2 NeuronCores.

Strategy (expert-parallel, sparse dispatch, per the sharding hint):
  - Router sharded over cores: core c computes fp32 logits for its 512-token
    shard on the TensorEngine (Wg stationary), AllGathers them (contiguous
    [E, tok] layout, transposed back on the PE) so every core holds identical
    logits for all 4096 tokens; softmax/top-2 on-device.
  - Core c owns expert c: slot positions come from matmul-based exclusive
    cumsums; the slot->token map is built with a single indirect-scatter DMA
    (token ids scattered to their slot positions; unrouted tokens get an
    out-of-bounds position and are dropped via bounds_check). Routed tokens
    are gathered via indirect DMA, transposed on the TensorEngine, and run
    through the two-layer FFN in bf16 (capacity padded to a multiple of 384).
  - Unscaled expert outputs (bf16) are AllGathered chunk-by-chunk into one
    shared [NG*8*CHK, D] tensor (overlapped with the FFN); each core combines
    the top-2 contributions for its own 512-token shard with two indirect
    row-gathers per 128-token tile + gate-weighted sum in fp32.

Numerics: router fp32 (top-2 selection fidelity), FFN bf16 with fp32
accumulation in PSUM, combine in fp32.
"""

import os
import sys

import numpy as np

for _p in ("/opt/trn_rl_repo",):
    if _p not in sys.path:
        sys.path.append(_p)

import ml_dtypes

import concourse.bass as bass
import concourse.mybir as mybir
import concourse.tile as tile
from concourse import bacc
from concourse.bass import IndirectOffsetOnAxis
from concourse.masks import make_identity

# Problem shapes (fixed per spec)
B, S, D, E = 2, 2048, 1024, 8
T = B * S          # 4096 tokens
F = 4 * D          # 4096 ffn dim
P = 128            # partitions
NT = T // P        # 32 token tiles
KD = D // P        # 8 contraction tiles over D
NF = F // P        # 32 f tiles
TOK_PER_CORE = T // E   # 512
OWN_TILES = TOK_PER_CORE // P  # 4
N_CORES = E
TG = 3                       # slot tiles per FFN group
CHK = TG * P                 # 384: slot chunk for the chunked AllGather
BIGPOS = float(1 << 20)      # scatter position for unrouted tokens (dropped)

f32 = mybir.dt.float32
bf16 = mybir.dt.bfloat16
f16 = mybir.dt.float16
i32 = mybir.dt.int32
u32 = mybir.dt.uint32

_cache = {}


def build_module(C: int, debug_out: bool = False):
    """Build the SPMD Bass module for capacity C (multiple of 384)."""
    assert C % CHK == 0
    ST = C // P  # slot tiles per expert

    nc = bacc.Bacc("TRN2", target_bir_lowering=False, debug=False,
                   num_devices=N_CORES)

    # ---- I/O ----
    xTs = nc.dram_tensor("xTs", [D, TOK_PER_CORE], f32,
                         kind="ExternalInput").ap()
    xbf = nc.dram_tensor("xbf", [T, D], bf16, kind="ExternalInput").ap()
    w1d = nc.dram_tensor("w1d", [D, F], bf16, kind="ExternalInput").ap()
    w2d = nc.dram_tensor("w2d", [F, D], bf16, kind="ExternalInput").ap()
    wgd = nc.dram_tensor("wgd", [D, E], f32, kind="ExternalInput").ap()
    bgb = nc.dram_tensor("bgb", [P, NT * E], f32, kind="ExternalInput").ap()
    b1pm = nc.dram_tensor("b1pm", [P, NF], f32, kind="ExternalInput").ap()
    b2pm = nc.dram_tensor("b2pm", [P, D], f32, kind="ExternalInput").ap()
    sel256 = nc.dram_tensor("sel256", [P, NT * E], f32,
                            kind="ExternalInput").ap()
    l128d = nc.dram_tensor("l128d", [P, P], f32, kind="ExternalInput").ap()
    ownmd = nc.dram_tensor("ownmd", [P, OWN_TILES * NT], f32,
                           kind="ExternalInput").ap()
    out = nc.dram_tensor("out", [TOK_PER_CORE, D], f32,
                         kind="ExternalOutput").ap()
    dbg = None
    if debug_out:
        dbg = {
            "dbg_l": nc.dram_tensor("dbg_l", [P, NT * E], f32,
                                    kind="ExternalOutput").ap(),
            "dbg_pos": nc.dram_tensor("dbg_pos", [P, NT * E], f32,
                                      kind="ExternalOutput").ap(),
            "dbg_posm": nc.dram_tensor("dbg_posm", [P, NT], f32,
                                       kind="ExternalOutput").ap(),
            "dbg_idx": nc.dram_tensor("dbg_idx", [P, C // P], i32,
                                      kind="ExternalOutput").ap(),
            "dbg_xgT": nc.dram_tensor("dbg_xgT", [P, C], bf16,
                                      kind="ExternalOutput").ap(),
            "dbg_y": nc.dram_tensor("dbg_y", [C, D], bf16,
                                    kind="ExternalOutput").ap(),
            "dbg_S": nc.dram_tensor("dbg_S", [P, 4 * (C // CHK) * NT], f32,
                                    kind="ExternalOutput").ap(),
        }

    with tile.TileContext(nc) as tc:
        _emit(tc, C, ST, xTs, xbf, w1d, w2d, wgd, bgb, b1pm, b2pm, sel256,
              l128d, ownmd, out, dbg)

    nc.compile()
    return nc


def _emit(tc, C, ST, xTs, xbf, w1d, w2d, wgd, bgb, b1pm, b2pm, sel256,
          l128d, ownmd, out, dbg=None):
    nc = tc.nc
    NE = NT * E  # 256
    NG = ST // TG  # number of FFN groups / y-AllGather chunks
    GBLK = N_CORES * CHK  # rows per AllGather chunk in y_big

    # ---------------- persistent pools ----------------
    persist = tc.alloc_tile_pool(name="persist", bufs=1)
    dram = tc.alloc_tile_pool(name="dram", bufs=1, space="DRAM")

    # constants / weights resident in SBUF
    wg_sb = persist.tile([P, KD, E], f32, name="wg_sb")
    nc.sync.dma_start(wg_sb[:], wgd.rearrange("(k p) e -> p k e", p=P))
    bg_sb = persist.tile([P, NE], f32, name="bg_sb")
    nc.sync.dma_start(bg_sb[:], bgb[:])
    sel_sb = persist.tile([P, NE], f32, name="sel_sb")
    nc.sync.dma_start(sel_sb[:], sel256[:])
    l128_sb = persist.tile([P, P], f32, name="l128_sb")
    nc.sync.dma_start(l128_sb[:], l128d[:])
    ownm_sb = persist.tile([P, OWN_TILES * NT], f32, name="ownm_sb")
    nc.sync.dma_start(ownm_sb[:], ownmd[:])
    b1_sb = persist.tile([P, NF], f32, name="b1_sb")
    nc.sync.dma_start(b1_sb[:], b1pm[:])
    b2_sb = persist.tile([P, D], f32, name="b2_sb")
    nc.sync.dma_start(b2_sb[:], b2pm[:])
    ident = persist.tile([P, P], bf16, name="ident")
    make_identity(nc, ident[:])
    identf = persist.tile([P, P], f32, name="identf")
    make_identity(nc, identf[:])
    ones_col = persist.tile([P, 1], f32, name="ones_col")
    nc.vector.memset(ones_col[:], 1.0)
    ones_row = persist.tile([1, P], f32, name="ones_row")
    nc.vector.memset(ones_row[:], 1.0)

    # w1/w2 tiles allocated here; the 16MB of loads are issued after the
    # router's small DMAs so they don't head-of-line block the critical path.
    # W2 is SBUF-resident so the FFN phase has no weight streaming competing
    # with the y-AllGather HBM traffic.
    w1_sb = [persist.tile([P, F], bf16, name=f"w1_sb{k}") for k in range(KD)]
    w2_sb = [persist.tile([P, D], bf16, name=f"w2_sb{f}") for f in range(NF)]

    # router / dispatch state kept for the combine phase
    exp_all = persist.tile([P, NE], f32, name="exp_all")    # exp(logits)
    m8_all = persist.tile([P, NE], f32, name="m8_all")      # per-tile top8 of exp
    r_all = persist.tile([P, NT], f32, name="r_all")        # 1/sum(exp)
    pos_all = persist.tile([P, NE], f32, name="pos_all")    # excl cumsum per expert
    ind_all = persist.tile([P, NE], f32, name="ind_all")    # top2 indicator
    ei_all = persist.tile([P, NE], u32, name="ei_all")      # top8 expert indices
    idx_sb = persist.tile([P, ST], i32, name="idx_sb")      # slot -> token id

    xgT = [persist.tile([P, C], bf16, name=f"xgT{d}") for d in range(KD)]

    l_dram = dram.tile([E, TOK_PER_CORE], f32, name="l_dram")
    lg_dram = dram.tile([N_CORES * E, TOK_PER_CORE], f32, addr_space="Shared",
                        name="lg_dram")
    y_dram = [dram.tile([CHK, D], bf16, name=f"y_dram{g}") for g in range(NG)]
    # one Shared AllGather output per chunk (Shared tensors allow one writer)
    y_all = [dram.tile([N_CORES * CHK, D], bf16, addr_space="Shared",
                       name=f"y_all{g}") for g in range(NG)]

    # ---------------- router (sharded + AllGather) ----------------
    with tc.tile_pool(name="router_sb", bufs=1, named_scope="router") as rpool, \
         tc.tile_pool(name="router_ps", bufs=1, space="PSUM") as rps:
        xs = rpool.tile([P, KD, TOK_PER_CORE], f32, name="xs")
        nc.sync.dma_start(xs[:], xTs.rearrange("(k p) t -> p k t", p=P))
        lT = rps.tile([E, TOK_PER_CORE], f32, name="lT")
        for k in range(KD):
            nc.tensor.matmul(lT[:], lhsT=wg_sb[:, k, :], rhs=xs[:, k, :],
                             start=(k == 0), stop=(k == KD - 1))
        lt_sb = rpool.tile([E, TOK_PER_CORE], f32, name="lt_sb")
        nc.vector.tensor_copy(lt_sb[:], lT[:])
        nc.sync.dma_start(l_dram[:], lt_sb[:])
        nc.gpsimd.collective_compute(
            "AllGather", mybir.AluOpType.bypass,
            replica_groups=[list(range(N_CORES))],
            ins=[l_dram[:].opt()], outs=[lg_dram[:].opt()],
        )
        # lg[(c e), tok_local]; token t*P+p has t = c*4 + t4,
        # tok_local = t4*P + p. Transpose each 128-token block on the PE.
        lg_sb = rpool.tile([N_CORES * E, TOK_PER_CORE], f32, name="lg_sb")
        nc.sync.dma_start(lg_sb[:], lg_dram[:])
        for k in range(KD):
            nc.sync.dma_start(w1_sb[k][:], w1d[k * P:(k + 1) * P, :])
        for f in range(NF):
            nc.sync.dma_start(w2_sb[f][:], w2d[f * P:(f + 1) * P, :])
        l_all = rpool.tile([P, NE], f32, name="l_all")
        l_all4 = l_all[:].rearrange("p (c t4 e) -> p c t4 e", c=N_CORES, t4=4)
        for t4 in range(4):
            ptp = rps.tile([P, N_CORES * E], f32, tag="ptp", bufs=2,
                           name="ptp")
            nc.tensor.transpose(ptp[:], lg_sb[:, t4 * P:(t4 + 1) * P],
                                identf[:N_CORES * E, :N_CORES * E])
            nc.vector.tensor_copy(
                l_all4[:, :, t4, :],
                ptp[:].rearrange("p (c e) -> p c e", e=E))
        nc.vector.tensor_add(l_all[:], l_all[:], bg_sb[:])
        if dbg is not None:
            nc.sync.dma_start(dbg["dbg_l"][:], l_all[:])
        nc.scalar.activation(exp_all[:], l_all[:],
                             mybir.ActivationFunctionType.Exp)
        # sums and reciprocal per token
        s_all = rpool.tile([P, NT], f32, name="s_all")
        nc.vector.reduce_sum(s_all[:], exp_all[:].rearrange(
            "p (t e) -> p t e", e=E), axis=mybir.AxisListType.X)
        nc.vector.reciprocal(r_all[:], s_all[:])
        # per-tile top8 (indices for the combine are extracted later, off
        # the dispatch critical path), then one batched top-2 indicator
        for tt in range(NT):
            sl = slice(tt * E, (tt + 1) * E)
            nc.vector.max(out=m8_all[:, sl], in_=exp_all[:, sl])
        m83r = m8_all[:].rearrange("p (t e) -> p t e", e=E)
        nc.vector.tensor_tensor(
            out=ind_all[:].rearrange("p (t e) -> p t e", e=E),
            in0=exp_all[:].rearrange("p (t e) -> p t e", e=E),
            in1=m83r[:, :, 1][:, :, None].to_broadcast([P, NT, E]),
            op=mybir.AluOpType.is_ge)

    # ---------------- dispatch: positions + scatter slot->token map --------
    with tc.tile_pool(name="disp_sb", bufs=1, named_scope="dispatch") as dpool, \
         tc.tile_pool(name="disp_ps", bufs=1, space="PSUM") as dps:
        # per-tile totals: ptot[0, (t e)] = sum_p ind_all[p, (t e)]
        ptot = dps.tile([1, NE], f32, name="ptot")
        nc.tensor.matmul(ptot[:], lhsT=ones_col[:], rhs=ind_all[:],
                         start=True, stop=True)
        tot_flat = dpool.tile([1, NE], f32, name="tot_flat")
        nc.vector.tensor_copy(tot_flat[:], ptot[:])
        # reshape [1, NT*E] -> [NT, E] via sbuf-to-sbuf DMA
        tot32 = dpool.tile([NT, E], f32, name="tot32")
        nc.sync.dma_start(tot32[:], tot_flat[:])
        # exclusive cumsum over tiles: strict-lower matmul
        pofs = dps.tile([NT, E], f32, name="pofs")
        nc.tensor.matmul(pofs[:], lhsT=l128_sb[:NT, :NT], rhs=tot32[:],
                         start=True, stop=True)
        ofs32 = dpool.tile([NT, E], f32, name="ofs32")
        nc.vector.tensor_copy(ofs32[:], pofs[:])
        ofs_flat = dpool.tile([1, NE], f32, name="ofs_flat")
        nc.sync.dma_start(ofs_flat[:], ofs32[:])
        # positions: local excl cumsum (over partitions) + tile offset
        ppos = dps.tile([P, NE], f32, name="ppos")
        nc.tensor.matmul(ppos[:], lhsT=l128_sb[:], rhs=ind_all[:],
                         start=True, stop=False)
        nc.tensor.matmul(ppos[:], lhsT=ones_row[:], rhs=ofs_flat[:],
                         start=False, stop=True)
        nc.vector.tensor_copy(pos_all[:], ppos[:])

        # my expert's positions / indicator
        tmp = dpool.tile([P, NE], f32, name="tmp")
        nc.vector.tensor_mul(tmp[:], pos_all[:], sel_sb[:])
        pos_e = dpool.tile([P, NT], f32, name="pos_e")
        nc.vector.reduce_sum(pos_e[:], tmp[:].rearrange(
            "p (t e) -> p t e", e=E), axis=mybir.AxisListType.X)
        nc.vector.tensor_mul(tmp[:], ind_all[:], sel_sb[:])
        ind_e = dpool.tile([P, NT], f32, name="ind_e")
        nc.vector.reduce_sum(ind_e[:], tmp[:].rearrange(
            "p (t e) -> p t e", e=E), axis=mybir.AxisListType.X)
        # masked positions: ind ? pos : BIGPOS (matches no slot)
        pos_m = dpool.tile([P, NT], f32, name="pos_m")
        nc.vector.tensor_scalar_add(pos_m[:], pos_e[:], -BIGPOS)
        nc.vector.tensor_mul(pos_m[:], pos_m[:], ind_e[:])
        nc.vector.tensor_scalar_add(pos_m[:], pos_m[:], BIGPOS)
        # remapped position pos2 = (pos & 127)*ST + (pos >> 7), so the final
        # departition DMA is contiguous per partition. BIGPOS remaps to
        # 8192 >= C (fp16-exact) and matches no slot.
        pos_i = dpool.tile([P, NT], i32, name="pos_i")
        nc.vector.tensor_copy(pos_i[:], pos_m[:])
        ph1 = dpool.tile([P, NT], i32, name="ph1")
        nc.vector.tensor_scalar(ph1[:], pos_i[:], 7, None,
                                op0=mybir.AluOpType.logical_shift_right)
        ph2 = dpool.tile([P, NT], i32, name="ph2")
        nc.vector.tensor_scalar(ph2[:], pos_i[:], 127, None,
                                op0=mybir.AluOpType.bitwise_and)
        nc.vector.tensor_scalar(ph2[:], ph2[:], ST, None,
                                op0=mybir.AluOpType.mult)
        pos2 = dpool.tile([P, NT], i32, name="pos2")
        nc.vector.tensor_add(pos2[:], ph1[:], ph2[:])
        pos_mh = dpool.tile([P, NT], f32, name="pos_mh")
        nc.vector.tensor_copy(pos_mh[:], pos2[:])

        # slot->token map via fp16 one-hot + rank-2 matmul:
        #   Pt[p, c] = (c == pos2[p, tt]);  token id = p + 128*tt, so
        #   accumulate [p-part; 128*tt-part] over tiles with a 2-col lhsT.
        # (These iota/const tiles have no deps and get scheduled early.)
        iotaC_i = dpool.tile([P, C], i32, name="iotaC_i")
        nc.gpsimd.iota(iotaC_i[:], pattern=[[1, C]], base=0,
                       channel_multiplier=0)
        iotaC_h = dpool.tile([P, C], f16, name="iotaC_h")
        nc.vector.tensor_copy(iotaC_h[:], iotaC_i[:])
        tokp_i = dpool.tile([P, 1], i32, name="tokp_i")
        nc.gpsimd.iota(tokp_i[:], pattern=[[0, 1]], base=0,
                       channel_multiplier=1)
        tokt_i = dpool.tile([P, NT], i32, name="tokt_i")
        nc.gpsimd.iota(tokt_i[:], pattern=[[P, NT]], base=0,
                       channel_multiplier=0)
        tok2 = dpool.tile([P, NT, 2], f16, name="tok2")
        nc.vector.tensor_copy(tok2[:, :, 0],
                              tokp_i[:, 0:1].to_broadcast([P, NT]))
        nc.vector.tensor_copy(tok2[:, :, 1], tokt_i[:])

        NCH = (C + 511) // 512
        pid_ps = [dps.tile([2, min(512, C - ch * 512)], f32,
                           name=f"pid{ch}") for ch in range(NCH)]
        for tt in range(NT):
            Pt = dpool.tile([P, C], f16, tag="Pt", bufs=6, name="Pt")
            nc.vector.tensor_scalar(Pt[:], iotaC_h[:],
                                    pos_mh[:, tt:tt + 1], None,
                                    op0=mybir.AluOpType.is_equal)
            for ch in range(NCH):
                c0 = ch * 512
                c1 = min(c0 + 512, C)
                nc.tensor.matmul(pid_ps[ch][:], lhsT=tok2[:, tt, :],
                                 rhs=Pt[:, c0:c1],
                                 start=(tt == 0), stop=(tt == NT - 1))
        pid_sb = dpool.tile([2, C], f32, name="pid_sb")
        for ch in range(NCH):
            c0 = ch * 512
            c1 = min(c0 + 512, C)
            nc.vector.tensor_copy(pid_sb[:, c0:c1], pid_ps[ch][:])
        # departition [2, C] -> [P, 2, ST] (contiguous per partition), then
        # token id = p-part + 128*tt-part
        idx2f = dpool.tile([P, 2, ST], f32, name="idx2f")
        for r in range(2):
            nc.sync.dma_start(idx2f[:, r, :], pid_sb[r:r + 1, :])
        idxf = dpool.tile([P, ST], f32, name="idxf")
        nc.vector.tensor_add(idxf[:], idx2f[:, 0, :], idx2f[:, 1, :])
        nc.vector.tensor_copy(idx_sb[:], idxf[:])
        if dbg is not None:
            nc.sync.dma_start(dbg["dbg_pos"][:], pos_all[:])
            nc.sync.dma_start(dbg["dbg_posm"][:], pos_m[:])
            nc.sync.dma_start(dbg["dbg_idx"][:], idx_sb[:])

        # gather tokens (bf16) and transpose into xgT
        with tc.tile_pool(name="gat_sb", bufs=3) as gpool, \
             tc.tile_pool(name="gat_ps", bufs=2, space="PSUM") as gps:
            for s in range(ST):
                xg = gpool.tile([P, D], bf16, tag="xg", name="xg")
                nc.gpsimd.indirect_dma_start(
                    out=xg[:], out_offset=None, in_=xbf[:],
                    in_offset=IndirectOffsetOnAxis(ap=idx_sb[:, s:s + 1],
                                                   axis=0),
                )
                for d in range(KD):
                    pt = gps.tile([P, P], bf16, tag="pt", name="pt")
                    nc.tensor.transpose(pt[:], xg[:, d * P:(d + 1) * P],
                                        ident[:])
                    # alternate the PSUM drains between DVE and ScalarE so
                    # neither engine gates the gather/transpose pipeline
                    if d % 2 == 0:
                        nc.vector.tensor_copy(
                            xgT[d][:, s * P:(s + 1) * P], pt[:])
                    else:
                        nc.scalar.activation(
                            xgT[d][:, s * P:(s + 1) * P], pt[:],
                            mybir.ActivationFunctionType.Copy)

    # ---------------- combine planes (needs only router/dispatch state) ----
    cpool = tc.alloc_tile_pool(name="comb_sb", bufs=1)
    with tc.tile_pool(name="comb_tmp", bufs=2, named_scope="combine") as ctmp:
        # top-8 indices (only top-2 used), off the dispatch critical path
        for tt in range(NT):
            sl = slice(tt * E, (tt + 1) * E)
            nc.vector.max_index(out=ei_all[:, sl], in_max=m8_all[:, sl],
                                in_values=exp_all[:, sl])
        # expert ids of top-1/top-2 as f32
        e1f = ctmp.tile([P, NT], f32, tag="e1f", bufs=1, name="e1f")
        e2f = ctmp.tile([P, NT], f32, tag="e2f", bufs=1, name="e2f")
        ei3 = ei_all[:].rearrange("p (t e) -> p t e", e=E)
        nc.vector.tensor_copy(e1f[:], ei3[:, :, 0])
        nc.vector.tensor_copy(e2f[:], ei3[:, :, 1])
        ioz = ctmp.tile([P, NE], i32, tag="ioz", bufs=1, name="ioz")
        nc.gpsimd.iota(ioz[:].rearrange("p (t e) -> p t e", e=E),
                       pattern=[[0, NT], [1, E]], base=0, channel_multiplier=0)
        iof = ctmp.tile([P, NE], f32, tag="iof", bufs=1, name="iof")
        nc.vector.tensor_copy(iof[:], ioz[:])
        m83 = m8_all[:].rearrange("p (t e) -> p t e", e=E)

        # Selection stack S: for each (q, g) a row-index plane into y_all[g]
        # (rows not in chunk g point at row 0) and a masked gate-weight
        # plane; one masked reduce per owner tile pulls all 12 values.
        NSEL = 2 * NG
        S = ctmp.tile([P, 2 * NSEL, NT], f32, bufs=1, name="S")
        for q, ef in ((0, e1f), (1, e2f)):
            oh = ctmp.tile([P, NE], f32, tag=f"oh{q}", bufs=1, name=f"oh{q}")
            nc.vector.tensor_tensor(
                out=oh[:].rearrange("p (t e) -> p t e", e=E),
                in0=iof[:].rearrange("p (t e) -> p t e", e=E),
                in1=ef[:, :, None].to_broadcast([P, NT, E]),
                op=mybir.AluOpType.is_equal)
            nc.vector.tensor_mul(oh[:], oh[:], pos_all[:])
            slot = ctmp.tile([P, NT], f32, tag=f"slot{q}", bufs=1,
                             name=f"slot{q}")
            nc.vector.reduce_sum(slot[:], oh[:].rearrange(
                "p (t e) -> p t e", e=E), axis=mybir.AxisListType.X)
            # chunk id g = (slot>=CHK) + (slot>=2*CHK) + ...
            gch = ctmp.tile([P, NT], f32, tag=f"gch{q}", bufs=1,
                            name=f"gch{q}")
            nc.vector.tensor_scalar(gch[:], slot[:], float(CHK), None,
                                    op0=mybir.AluOpType.is_ge)
            for gg in range(2, NG):
                t2 = ctmp.tile([P, NT], f32, tag="t2", name="t2")
                nc.vector.tensor_scalar(t2[:], slot[:], float(CHK * gg), None,
                                        op0=mybir.AluOpType.is_ge)
                nc.vector.tensor_add(gch[:], gch[:], t2[:])
            # in-chunk row: e*CHK + (slot - g*CHK)
            base = ctmp.tile([P, NT], f32, tag=f"base{q}", bufs=1,
                             name=f"base{q}")
            nc.vector.scalar_tensor_tensor(
                out=base[:], in0=ef[:], scalar=float(CHK), in1=slot[:],
                op0=mybir.AluOpType.mult, op1=mybir.AluOpType.add)
            gv = ctmp.tile([P, NT], f32, tag=f"gv{q}", bufs=1, name=f"gv{q}")
            nc.vector.tensor_tensor(out=gv[:], in0=m83[:, :, q], in1=r_all[:],
                                    op=mybir.AluOpType.mult)
            for gg in range(NG):
                k = q * NG + gg
                eq = ctmp.tile([P, NT], f32, tag="eq", name="eq")
                nc.vector.tensor_scalar(eq[:], gch[:], float(gg), None,
                                        op0=mybir.AluOpType.is_equal)
                nc.vector.tensor_scalar_add(S[:, k, :], base[:],
                                            float(-CHK * gg))
                nc.vector.tensor_mul(S[:, k, :], S[:, k, :], eq[:])
                nc.vector.tensor_mul(S[:, NSEL + k, :], eq[:], gv[:])

        if dbg is not None:
            nc.sync.dma_start(
                dbg["dbg_S"][:].rearrange("p (k t) -> p k t", k=2 * NSEL),
                S[:])
        # per-owner-tile row indices and weights, ready before the FFN
        reds = []
        redis = []
        for j in range(OWN_TILES):
            own = ownm_sb[:, j * NT:(j + 1) * NT]
            tmpS = ctmp.tile([P, 2 * NSEL, NT], f32, tag="tmpS", bufs=2,
                             name="tmpS")
            nc.vector.tensor_tensor(
                out=tmpS[:], in0=S[:],
                in1=own[:, None, :].to_broadcast([P, 2 * NSEL, NT]),
                op=mybir.AluOpType.mult)
            red = cpool.tile([P, 2 * NSEL], f32, name=f"red{j}")
            nc.vector.reduce_sum(red[:], tmpS[:], axis=mybir.AxisListType.X)
            redi = cpool.tile([P, NSEL], i32, name=f"redi{j}")
            nc.vector.tensor_copy(redi[:], red[:, :NSEL])
            reds.append(red)
            redis.append(redi)
    ots = [cpool.tile([P, D], f32, name=f"ot{j}") for j in range(OWN_TILES)]

    # ---------------- FFN (bf16) + chunked y AllGather + combine ----------
    def emit_combine(g):
        # gather+accumulate chunk g's contributions for the own shard
        with tc.tile_pool(name=f"comb_g{g}", bufs=2,
                          named_scope="combine") as cg:
            for j in range(OWN_TILES):
                for q in range(2):
                    k = q * NG + g
                    yt = cg.tile([P, D], bf16, tag="yt", bufs=4, name="yt")
                    nc.gpsimd.indirect_dma_start(
                        out=yt[:], out_offset=None, in_=y_all[g][:],
                        in_offset=IndirectOffsetOnAxis(
                            ap=redis[j][:, k:k + 1], axis=0))
                    w = reds[j][:, NSEL + k:NSEL + k + 1]
                    if g == 0 and q == 0:
                        nc.vector.tensor_scalar(
                            ots[j][:], yt[:], w, None,
                            op0=mybir.AluOpType.mult)
                    else:
                        nc.vector.scalar_tensor_tensor(
                            out=ots[j][:], in0=yt[:], scalar=w,
                            in1=ots[j][:], op0=mybir.AluOpType.mult,
                            op1=mybir.AluOpType.add)

    with tc.tile_pool(name="ffn_sb", bufs=1, named_scope="ffn") as fpool, \
         tc.tile_pool(name="ffn_ps", bufs=1, space="PSUM") as fps:
        for g in range(NG):
            t0 = g * TG
            py = [[fps.tile([P, 512], f32, tag=f"py_{t}_{n}",
                            name=f"py_{t}_{n}")
                   for n in range(2)] for t in range(TG)]
            for f in range(NF):
                ph = fps.tile([P, CHK], f32, tag="ph", bufs=2, name="ph")
                for k in range(KD):
                    nc.tensor.matmul(
                        ph[:], lhsT=w1_sb[k][:, f * P:(f + 1) * P],
                        rhs=xgT[k][:, t0 * P:t0 * P + CHK],
                        start=(k == 0), stop=(k == KD - 1))
                hbuf = fpool.tile([P, CHK], bf16, tag="hbuf", bufs=3,
                                  name="hbuf")
                nc.scalar.activation(hbuf[:], ph[:],
                                     mybir.ActivationFunctionType.Relu,
                                     bias=b1_sb[:, f:f + 1], scale=1.0)
                for t in range(TG):
                    for n in range(2):
                        nc.tensor.matmul(
                            py[t][n][:],
                            lhsT=hbuf[:, t * P:(t + 1) * P],
                            rhs=w2_sb[f][:, n * 512:(n + 1) * 512],
                            start=(f == 0), stop=(f == NF - 1))
            # add b2 (replicated across partitions) during PSUM drain
            for t in range(TG):
                ysb = fpool.tile([P, D], bf16, tag="ysb", bufs=2, name="ysb")
                for n in range(2):
                    nc.vector.tensor_tensor(
                        out=ysb[:, n * 512:(n + 1) * 512], in0=py[t][n][:],
                        in1=b2_sb[:, n * 512:(n + 1) * 512],
                        op=mybir.AluOpType.add)
                nc.sync.dma_start(y_dram[g][t * P:(t + 1) * P, :], ysb[:])
            # combine the PREVIOUS chunk (its AllGather has landed by now)
            # before this chunk's AG trigger, so the Pool queue isn't blocked
            if g >= 1:
                emit_combine(g - 1)
            # ship this chunk while the next group computes
            nc.gpsimd.collective_compute(
                "AllGather", mybir.AluOpType.bypass,
                replica_groups=[list(range(N_CORES))],
                ins=[y_dram[g][:].opt()],
                outs=[y_all[g][:].opt()],
            )
        emit_combine(NG - 1)
        if dbg is not None:
            nc.sync.dma_start(dbg["dbg_xgT"][:], xgT[0][:])
            for g in range(NG):
                nc.sync.dma_start(dbg["dbg_y"][g * CHK:(g + 1) * CHK, :],
                                  y_dram[g][:])

    for j in range(OWN_TILES):
        nc.sync.dma_start(out[j * P:(j + 1) * P, :], ots[j][:])

    cpool.release()
    persist.release()
    dram.release()


def _host_prep(x, Wg, bg, W1, b1, W2, b2, C):
    xf = np.ascontiguousarray(x.reshape(T, D).astype(np.float32))
    xT = np.ascontiguousarray(xf.T)
    xbf = xf.astype(ml_dtypes.bfloat16)
    bgb = np.tile(bg.astype(np.float32), NT)[None, :].repeat(P, 0)
    bgb = np.ascontiguousarray(bgb)
    l128 = np.triu(np.ones((P, P), np.float32), 1)  # [t', t] = 1 if t' < t
    in_maps = []
    for c in range(N_CORES):
        sel = np.zeros(E, np.float32)
        sel[c] = 1.0
        sel256 = np.ascontiguousarray(np.tile(sel, NT)[None, :].repeat(P, 0))
        ownm = np.zeros((P, OWN_TILES, NT), np.float32)
        for j in range(OWN_TILES):
            ownm[:, j, OWN_TILES * c + j] = 1.0
        in_maps.append({
            "xTs": np.ascontiguousarray(
                xT[:, c * TOK_PER_CORE:(c + 1) * TOK_PER_CORE]),
            "xbf": xbf,
            "w1d": np.ascontiguousarray(W1[c].astype(ml_dtypes.bfloat16)),
            "w2d": np.ascontiguousarray(W2[c].astype(ml_dtypes.bfloat16)),
            "wgd": np.ascontiguousarray(Wg.astype(np.float32)),
            "bgb": bgb,
            "b1pm": np.ascontiguousarray(
                b1[c].astype(np.float32).reshape(NF, P).T),
            "b2pm": np.ascontiguousarray(
                np.tile(b2[c].astype(np.float32)[None, :], (P, 1))),
            "sel256": sel256,
            "l128d": l128,
            "ownmd": np.ascontiguousarray(ownm.reshape(P, OWN_TILES * NT)),
        })
    return in_maps


def _capacity(x, Wg, bg):
    xf = x.reshape(T, D).astype(np.float32)
    logits = xf @ Wg.astype(np.float32) + bg.astype(np.float32)
    part = np.partition(logits, E - 2, axis=-1)
    m2 = part[:, E - 2:E - 1]
    counts = (logits >= m2).sum(0)
    return int(np.ceil((counts.max() + 16) / CHK) * CHK)


LAST_RESULT = None


def kernel(x, Wg, bg, W1, b1, W2, b2):
    global LAST_RESULT
    from concourse.bass_utils import run_bass_kernel_spmd

    x = np.asarray(x)
    C = _capacity(x, np.asarray(Wg), np.asarray(bg))
    if C not in _cache:
        _cache[C] = build_module(C)
    nc = _cache[C]
    in_maps = _host_prep(x, np.asarray(Wg), np.asarray(bg), np.asarray(W1),
                         np.asarray(b1), np.asarray(W2), np.asarray(b2), C)
    trace = bool(os.environ.get("BASS_TRACE"))
    if trace:
        _setup_axon_profile_hook()
    res = run_bass_kernel_spmd(nc, in_maps, core_ids=list(range(N_CORES)),
                               trace=trace)
    LAST_RESULT = res
    out = np.empty((T, D), np.float32)
    for c in range(N_CORES):
        out[c * TOK_PER_CORE:(c + 1) * TOK_PER_CORE] = res.results[c]["out"]
    return out.reshape(B, S, D)


def _setup_axon_profile_hook():
    """Provide antenv.axon_hooks (missing in this image) so trace=True works."""
    import types
    try:
        import antenv
        if "antenv.axon_hooks" not in sys.modules:
            hooks = types.ModuleType("antenv.axon_hooks")
            hooks._hook = None
            hooks.set_axon_ntff_profile_hook = \
                lambda h: setattr(hooks, "_hook", h)
            hooks.get_axon_ntff_profile_hook = lambda: hooks._hook
            sys.modules["antenv.axon_hooks"] = hooks
            antenv.axon_hooks = hooks
            from trn_agent_boot.trn_boot import _ntff_profile_via_ctypes
            hooks.set_axon_ntff_profile_hook(
                _ntff_profile_via_ctypes("/opt/axon/libaxon_pjrt.so"))
    except Exception as e:  # profiling is best-effort
        print(f"profile hook setup failed: {e}", file=sys.stderr)
